# revision 4
# baseline (speedup 1.0000x reference)
# Trainium2 Bass kernel for NonLocalBlock (GroupNorm + 1x1-conv self-attention + residual).
#
# Full input x: [4, 256, 64, 64] f32. Output: x + proj(attn(gn(x))), same shape.
#
# Sharding: 8 cores = 4 batches x 2 query-halves. Attention is independent per
# batch; within a batch, softmax rows (queries) split cleanly across 2 cores.
# Each core redundantly computes GroupNorm + K + V^T for its batch (cheap), and
# computes scores/softmax/PV/proj only for its 2048 queries. No collectives.
#
# Per-core program layout (c = 256 channels as 2 partition-tiles, n = 4096):
#   - GroupNorm stats: bn_stats/bn_aggr per channel, group-combine and
#     broadcast-back via tiny PE matmuls with 0/1 group matrices.
#   - h = x*A + B (bf16), also on the query half (separate input slice, so all
#     access patterns stay static across the SPMD program).
#   - k[c,j] (bf16), q[c,i] (bf16, 1/sqrt(c) folded into wq on host),
#     vT[j,c] (bf16, computed directly in transposed layout).
#   - scores transposed: sT[j,i] = k^T q via PE; exp on ACT fused with the
#     PSUM->SBUF copy; eT[j,i] is then directly the PV moving operand.
#   - row sums of exp via ones-vector PE matmuls; softmax normalization is a
#     column scaling that commutes through PV and proj, applied at the end.
#   - bv never applied on-chip: softmax rows sum to 1, so wproj@bv folds into
#     bproj on the host.
#   - out = x_half + rinv * (wproj @ A_unnorm) + bproj_eff.

import os
import sys

for _p in ("/opt/trn_rl_repo", "/root/.axon_site/_ro/trn_rl_repo"):
    if os.path.isdir(_p) and _p not in sys.path:
        sys.path.insert(0, _p)

import numpy as np
import ml_dtypes

import concourse.bass as bass
import concourse.tile as tile
from concourse import bacc, mybir
from concourse.alu_op_type import AluOpType
from concourse.bass_utils import run_bass_kernel_spmd

F32 = mybir.dt.float32
BF16 = mybir.dt.bfloat16
AF = mybir.ActivationFunctionType

B = 4
C = 256
N = 4096           # 64*64 spatial positions
NH = N // 2        # queries per core
GROUPS = 32
GSIZE = C // GROUPS  # 8 channels per group
EPS = 1e-6
P = 128
CT = C // P        # 2 channel tiles
JT = N // P        # 32 key tiles
NB = NH // 512     # 4 query blocks of 512
NCORES = 8

_cache = {}


def _col(ap_1d, ct):
    # View a [256] DRAM tensor as [256, 1] and take channel-tile ct's [128, 1].
    return ap_1d.ap().rearrange("(a b) -> a b", b=1)[ct * P:(ct + 1) * P, :]


def _build_program():
    nc = bacc.Bacc("TRN2", target_bir_lowering=False, debug=False)

    x_full = nc.dram_tensor("x_full", [C, N], F32, kind="ExternalInput")
    xh = nc.dram_tensor("xh", [C, NH], F32, kind="ExternalInput")
    gnsc = nc.dram_tensor("gnsc", [C], F32, kind="ExternalInput")
    gnbs = nc.dram_tensor("gnbs", [C], F32, kind="ExternalInput")
    g8 = nc.dram_tensor("g8", [P, P // GSIZE], F32, kind="ExternalInput")
    gt01 = nc.dram_tensor("gt01", [P // GSIZE, P], F32, kind="ExternalInput")
    wqT = nc.dram_tensor("wqT", [C, C], BF16, kind="ExternalInput")
    bq = nc.dram_tensor("bq", [C], F32, kind="ExternalInput")
    wkT = nc.dram_tensor("wkT", [C, C], BF16, kind="ExternalInput")
    bk = nc.dram_tensor("bk", [C], F32, kind="ExternalInput")
    wvT = nc.dram_tensor("wvT", [C, C], BF16, kind="ExternalInput")
    wpT = nc.dram_tensor("wpT", [C, C], BF16, kind="ExternalInput")
    bpe = nc.dram_tensor("bpe", [C], F32, kind="ExternalInput")
    out = nc.dram_tensor("out", [C, NH], F32, kind="ExternalOutput")
    rinv_scr = nc.dram_tensor("rinv_scr", [NH], F32)

    with tile.TileContext(nc) as tc:
        _body(tc, x_full, xh, gnsc, gnbs, g8, gt01,
              wqT, bq, wkT, bk, wvT, wpT, bpe, out, rinv_scr)
    nc.compile()
    return nc


def _body(tc, x_full, xh, gnsc, gnbs, g8, gt01,
          wqT, bq, wkT, bk, wvT, wpT, bpe, out, rinv_scr):
    nc = tc.nc
    NG = P // GSIZE  # 16 groups per channel tile

    from contextlib import ExitStack
    with ExitStack() as ctx:
        consts = ctx.enter_context(tc.tile_pool(name="consts", bufs=1))
        px = ctx.enter_context(tc.tile_pool(name="px", bufs=1))
        ph = ctx.enter_context(tc.tile_pool(name="ph", bufs=1))
        pkv = ctx.enter_context(tc.tile_pool(name="pkv", bufs=1))
        pet = ctx.enter_context(tc.tile_pool(name="pet", bufs=2))
        pst = ctx.enter_context(tc.tile_pool(name="pst", bufs=4))
        pout = ctx.enter_context(tc.tile_pool(name="pout", bufs=3))
        ps_sc = ctx.enter_context(tc.tile_pool(name="ps_sc", bufs=2, space="PSUM"))
        ps_sum = ctx.enter_context(tc.tile_pool(name="ps_sum", bufs=2, space="PSUM"))
        ps_pv = ctx.enter_context(tc.tile_pool(name="ps_pv", bufs=2, space="PSUM"))
        ps_misc = ctx.enter_context(tc.tile_pool(name="ps_misc", bufs=2, space="PSUM"))

        # ---- constants ----
        ones_b = consts.tile([P, 1], BF16, tag="ones")
        nc.vector.memset(ones_b, 1.0)
        g8_sb = consts.tile([P, NG], F32, tag="g8")
        nc.sync.dma_start(out=g8_sb, in_=g8.ap())
        gt01_sb = consts.tile([NG, P], F32, tag="gt01")
        nc.sync.dma_start(out=gt01_sb, in_=gt01.ap())

        w_sb = {}
        for name, h in (("wqT", wqT), ("wkT", wkT), ("wvT", wvT), ("wpT", wpT)):
            for ec in range(CT):
                t = consts.tile([P, C], BF16, tag=f"{name}{ec}")
                nc.sync.dma_start(out=t, in_=h.ap()[ec * P:(ec + 1) * P, :])
                w_sb[(name, ec)] = t

        col_sb = {}
        for name, h in (("gnsc", gnsc), ("gnbs", gnbs), ("bq", bq),
                        ("bk", bk), ("bpe", bpe)):
            for ct in range(CT):
                t = consts.tile([P, 1], F32, tag=f"{name}{ct}")
                nc.sync.dma_start(out=t, in_=_col(h, ct))
                col_sb[(name, ct)] = t

        # ---- load x, GroupNorm stats, normalize ----
        x_sb = []
        for ct in range(CT):
            xt = px.tile([P, N], F32, tag=f"x{ct}")
            for c4 in range(4):
                nc.sync.dma_start(
                    out=xt[:, c4 * 1024:(c4 + 1) * 1024],
                    in_=x_full.ap()[ct * P:(ct + 1) * P, c4 * 1024:(c4 + 1) * 1024])
            x_sb.append(xt)

        h_sb, hh_sb, ab_cols = [], [], []
        for ct in range(CT):
            xt = x_sb[ct]
            stats = pst.tile([P, 8, nc.vector.BN_STATS_DIM], F32, tag="bnst")
            for s in range(8):
                nc.vector.bn_stats(out=stats[:, s, :], in_=xt[:, s * 512:(s + 1) * 512])
            mv = pst.tile([P, nc.vector.BN_AGGR_DIM], F32, tag="bnagg")
            nc.vector.bn_aggr(out=mv, in_=stats)

            # per-channel (mean, E[x^2]) -> per-group via G/8 matmul
            st2 = pst.tile([P, 2], F32, tag="st2")
            nc.vector.tensor_copy(out=st2[:, 0:1], in_=mv[:, 0:1])
            m2 = pst.tile([P, 1], F32, tag="m2")
            nc.vector.tensor_mul(m2, mv[:, 0:1], mv[:, 0:1])
            nc.vector.tensor_add(st2[:, 1:2], m2, mv[:, 1:2])

            gps = ps_misc.tile([NG, 2], F32, tag="m")
            nc.tensor.matmul(gps, lhsT=g8_sb, rhs=st2, start=True, stop=True)
            gs = pst.tile([NG, 2], F32, tag="gs")
            nc.vector.tensor_copy(out=gs, in_=gps)

            # var_g = E[x^2]_g - mean_g^2 ; rstd = 1/sqrt(var+eps)
            vg = pst.tile([NG, 1], F32, tag="vg")
            nc.vector.tensor_mul(vg, gs[:, 0:1], gs[:, 0:1])
            nc.vector.tensor_tensor(out=vg, in0=gs[:, 1:2], in1=vg,
                                    op=AluOpType.subtract)
            eps_t = pst.tile([NG, 1], F32, tag="eps")
            nc.vector.memset(eps_t, EPS)
            std = pst.tile([NG, 1], F32, tag="std")
            nc.scalar.activation(out=std, in_=vg, func=AF.Sqrt, bias=eps_t, scale=1.0)
            rstd = pst.tile([NG, 1], F32, tag="rstd")
            nc.vector.reciprocal(out=rstd, in_=std)

            gs2 = pst.tile([NG, 2], F32, tag="gs2")
            nc.vector.tensor_copy(out=gs2[:, 0:1], in_=gs[:, 0:1])
            nc.vector.tensor_copy(out=gs2[:, 1:2], in_=rstd)

            # broadcast (mean_g, rstd_g) back to channels
            bps = ps_misc.tile([P, 2], F32, tag="m")
            nc.tensor.matmul(bps, lhsT=gt01_sb, rhs=gs2, start=True, stop=True)
            mr = pst.tile([P, 2], F32, tag="mr")
            nc.vector.tensor_copy(out=mr, in_=bps)

            a_col = pst.tile([P, 1], F32, tag=f"acol{ct}")
            nc.vector.tensor_mul(a_col, mr[:, 1:2], col_sb[("gnsc", ct)])
            b_col = pst.tile([P, 1], F32, tag=f"bcol{ct}")
            nc.vector.tensor_mul(b_col, mr[:, 0:1], a_col)
            nc.vector.tensor_tensor(out=b_col, in0=col_sb[("gnbs", ct)],
                                    in1=b_col, op=AluOpType.subtract)
            ab_cols.append((a_col, b_col))

            ht = ph.tile([P, N], BF16, tag=f"h{ct}")
            nc.vector.tensor_scalar(out=ht, in0=xt, scalar1=a_col, scalar2=b_col,
                                    op0=AluOpType.mult, op1=AluOpType.add)
            h_sb.append(ht)

        # query-half of h (separate input keeps APs static across cores)
        xh_sb = []
        for ct in range(CT):
            xht = px.tile([P, NH], F32, tag=f"x{ct}")
            nc.sync.dma_start(out=xht, in_=xh.ap()[ct * P:(ct + 1) * P, :])
            xh_sb.append(xht)
            a_col, b_col = ab_cols[ct]
            hht = ph.tile([P, NH], BF16, tag=f"hh{ct}")
            nc.vector.tensor_scalar(out=hht, in0=xht, scalar1=a_col, scalar2=b_col,
                                    op0=AluOpType.mult, op1=AluOpType.add)
            hh_sb.append(hht)

        # ---- projections ----
        k_sb = [pkv.tile([P, N], BF16, tag=f"k{dt}", name=f"k{dt}") for dt in range(CT)]
        for dt in range(CT):
            for jc in range(N // 512):
                ps = ps_misc.tile([P, 512], F32, tag="m")
                for ec in range(CT):
                    nc.tensor.matmul(
                        ps, lhsT=w_sb[("wkT", ec)][:, dt * P:(dt + 1) * P],
                        rhs=h_sb[ec][:, jc * 512:(jc + 1) * 512],
                        start=(ec == 0), stop=(ec == CT - 1))
                nc.vector.tensor_scalar(
                    out=k_sb[dt][:, jc * 512:(jc + 1) * 512], in0=ps,
                    scalar1=col_sb[("bk", dt)], scalar2=None, op0=AluOpType.add)

        q_sb = [pkv.tile([P, NH], BF16, tag=f"q{dt}", name=f"q{dt}") for dt in range(CT)]
        for dt in range(CT):
            for ic in range(NH // 512):
                ps = ps_misc.tile([P, 512], F32, tag="m")
                for ec in range(CT):
                    nc.tensor.matmul(
                        ps, lhsT=w_sb[("wqT", ec)][:, dt * P:(dt + 1) * P],
                        rhs=hh_sb[ec][:, ic * 512:(ic + 1) * 512],
                        start=(ec == 0), stop=(ec == CT - 1))
                nc.vector.tensor_scalar(
                    out=q_sb[dt][:, ic * 512:(ic + 1) * 512], in0=ps,
                    scalar1=col_sb[("bq", dt)], scalar2=None, op0=AluOpType.add)

        vT_sb = pkv.tile([P, JT, C], BF16, tag="vT")
        for jt in range(JT):
            ps = ps_misc.tile([P, 512], F32, tag="m")
            for ec in range(CT):
                nc.tensor.matmul(
                    ps[:, 0:C], lhsT=h_sb[ec][:, jt * P:(jt + 1) * P],
                    rhs=w_sb[("wvT", ec)],
                    start=(ec == 0), stop=(ec == CT - 1))
            nc.vector.tensor_copy(out=vT_sb[:, jt, :], in_=ps[:, 0:C])

        # ---- attention ----
        A_sb = [pkv.tile([P, NH], BF16, tag=f"A{ct}", name=f"A{ct}") for ct in range(CT)]
        sums_row = pkv.tile([1, NH], F32, tag="sums")
        for ib in range(NB):
            i0 = ib * 512
            eT = pet.tile([P, JT, 512], BF16, tag="eT")
            for jt in range(JT):
                ps = ps_sc.tile([P, 512], F32, tag="sc")
                for cc in range(CT):
                    nc.tensor.matmul(
                        ps, lhsT=k_sb[cc][:, jt * P:(jt + 1) * P],
                        rhs=q_sb[cc][:, i0:i0 + 512],
                        start=(cc == 0), stop=(cc == CT - 1))
                nc.scalar.activation(out=eT[:, jt, :], in_=ps, func=AF.Exp)

            pss = ps_sum.tile([1, 512], F32, tag="sm")
            for jt in range(JT):
                nc.tensor.matmul(pss, lhsT=ones_b, rhs=eT[:, jt, :],
                                 start=(jt == 0), stop=(jt == JT - 1))
            nc.vector.tensor_copy(out=sums_row[:, i0:i0 + 512], in_=pss)

            for ct in range(CT):
                psa = ps_pv.tile([P, 512], F32, tag="pv")
                for jt in range(JT):
                    nc.tensor.matmul(
                        psa, lhsT=vT_sb[:, jt, ct * P:(ct + 1) * P],
                        rhs=eT[:, jt, :],
                        start=(jt == 0), stop=(jt == JT - 1))
                nc.scalar.activation(out=A_sb[ct][:, i0:i0 + 512], in_=psa,
                                     func=AF.Copy)

        # ---- softmax normalization scale, broadcast via DRAM bounce ----
        rrow = pkv.tile([1, NH], F32, tag="rrow")
        nc.vector.reciprocal(out=rrow, in_=sums_row)
        nc.sync.dma_start(out=rinv_scr.ap().rearrange("(a b) -> a b", a=1),
                          in_=rrow)
        rinvb = pkv.tile([P, NH], F32, tag="rinvb")
        rsc = rinv_scr.ap()
        nc.sync.dma_start(
            out=rinvb,
            in_=bass.AP(tensor=rsc.tensor, offset=rsc.offset,
                        ap=[[0, P]] + [list(d) for d in rsc.ap]))

        # ---- output projection + normalization + bias + residual ----
        for dt in range(CT):
            for ic in range(NH // 512):
                ps = ps_misc.tile([P, 512], F32, tag="m")
                for cc in range(CT):
                    nc.tensor.matmul(
                        ps, lhsT=w_sb[("wpT", cc)][:, dt * P:(dt + 1) * P],
                        rhs=A_sb[cc][:, ic * 512:(ic + 1) * 512],
                        start=(cc == 0), stop=(cc == CT - 1))
                ot = pout.tile([P, 512], F32, tag="ot")
                nc.vector.tensor_mul(ot, ps, rinvb[:, ic * 512:(ic + 1) * 512])
                nc.vector.tensor_scalar(out=ot, in0=ot,
                                        scalar1=col_sb[("bpe", dt)],
                                        scalar2=None, op0=AluOpType.add)
                nc.vector.tensor_add(ot, ot, xh_sb[dt][:, ic * 512:(ic + 1) * 512])
                nc.sync.dma_start(
                    out=out.ap()[dt * P:(dt + 1) * P, ic * 512:(ic + 1) * 512],
                    in_=ot)


def _get_program():
    if "nc" not in _cache:
        _cache["nc"] = _build_program()
    return _cache["nc"]


def kernel(x, gn_scale, gn_bias, wq, bq, wk, bk, wv, bv, wproj, bproj):
    x = np.asarray(x, dtype=np.float32)
    b, c, hh, ww = x.shape
    assert (b, c, hh * ww) == (B, C, N)
    xf = np.ascontiguousarray(x.reshape(B, C, N))

    bf = ml_dtypes.bfloat16
    wqT_s = np.ascontiguousarray((np.asarray(wq, np.float32).T / np.sqrt(C))).astype(bf)
    bq_s = (np.asarray(bq, np.float32) / np.sqrt(C)).astype(np.float32)
    wkT = np.ascontiguousarray(np.asarray(wk, np.float32).T).astype(bf)
    wvT = np.ascontiguousarray(np.asarray(wv, np.float32).T).astype(bf)
    wpT = np.ascontiguousarray(np.asarray(wproj, np.float32).T).astype(bf)
    # softmax rows sum to 1 => v-bias contributes wproj@bv, constant per channel
    bpe = (np.asarray(bproj, np.float64)
           + np.asarray(wproj, np.float64) @ np.asarray(bv, np.float64)
           ).astype(np.float32)

    g8 = np.zeros((P, P // GSIZE), np.float32)
    gt01 = np.zeros((P // GSIZE, P), np.float32)
    for ch in range(P):
        g8[ch, ch // GSIZE] = 1.0 / (GSIZE * N)
        gt01[ch // GSIZE, ch] = 1.0
    # g8 entries 1/(8*4096): the G-matmul then yields per-group means directly
    # (bn per-channel stats are means over 4096, so scale by 4096/(8*4096)=1/8)
    g8 *= N

    common = dict(gnsc=np.asarray(gn_scale, np.float32),
                  gnbs=np.asarray(gn_bias, np.float32),
                  g8=g8, gt01=gt01,
                  wqT=wqT_s, bq=bq_s, wkT=wkT,
                  bk=np.asarray(bk, np.float32),
                  wvT=wvT, wpT=wpT, bpe=bpe)

    in_maps = []
    for core in range(NCORES):
        bi, half = core // 2, core % 2
        in_maps.append(dict(
            x_full=np.ascontiguousarray(xf[bi]),
            xh=np.ascontiguousarray(xf[bi][:, half * NH:(half + 1) * NH]),
            **common))

    nc = _get_program()
    trace = bool(os.environ.get("BASS_KERNEL_TRACE"))
    res = run_bass_kernel_spmd(nc, in_maps, core_ids=list(range(NCORES)),
                               trace=trace)
    _cache["last_results"] = res

    full = np.empty((B, C, N), np.float32)
    for core in range(NCORES):
        bi, half = core // 2, core % 2
        full[bi][:, half * NH:(half + 1) * NH] = res.results[core]["out"]
    return full.reshape(B, C, hh, ww)


# revision 9
# speedup vs baseline: 1.0645x; 1.0645x over previous
# Trainium2 Bass kernel for NonLocalBlock (GroupNorm + 1x1-conv self-attention + residual).
#
# Full input x: [4, 256, 64, 64] f32. Output: x + proj(attn(gn(x))), same shape.
#
# Sharding: 8 cores = 4 batches x 2 query-halves. Attention is independent per
# batch; within a batch, softmax rows (queries) split cleanly across 2 cores.
# Each core redundantly computes GroupNorm + K + V^T for its batch (cheap), and
# computes scores/softmax/PV/proj only for its 2048 queries. No collectives.
#
# Per-core program layout (c = 256 channels as 2 partition-tiles, n = 4096):
#   - GroupNorm stats: bn_stats/bn_aggr per channel, group-combine and
#     broadcast-back via tiny PE matmuls with 0/1 group matrices.
#   - h = x*A + B (bf16), also on the query half (separate input slice, so all
#     access patterns stay static across the SPMD program).
#   - k[c,j] (bf16), q[c,i] (bf16, 1/sqrt(c) folded into wq on host),
#     vT[j,c] (bf16, computed directly in transposed layout).
#   - scores transposed: sT[j,i] = k^T q via PE; exp on ACT fused with the
#     PSUM->SBUF copy; eT[j,i] is then directly the PV moving operand.
#   - row sums of exp via ones-vector PE matmuls; softmax normalization is a
#     column scaling that commutes through PV and proj, applied at the end.
#   - bv never applied on-chip: softmax rows sum to 1, so wproj@bv folds into
#     bproj on the host.
#   - out = x_half + rinv * (wproj @ A_unnorm) + bproj_eff.

import os
import sys

for _p in ("/opt/trn_rl_repo", "/root/.axon_site/_ro/trn_rl_repo"):
    if os.path.isdir(_p) and _p not in sys.path:
        sys.path.insert(0, _p)

import numpy as np
import ml_dtypes

import concourse.bass as bass
import concourse.tile as tile
from concourse import bacc, mybir
from concourse.alu_op_type import AluOpType
from concourse.bass_utils import run_bass_kernel_spmd

F32 = mybir.dt.float32
BF16 = mybir.dt.bfloat16
AF = mybir.ActivationFunctionType

B = 4
C = 256
N = 4096           # 64*64 spatial positions
NH = N // 2        # queries per core
GROUPS = 32
GSIZE = C // GROUPS  # 8 channels per group
EPS = 1e-6
P = 128
CT = C // P        # 2 channel tiles
JT = N // P        # 32 key tiles
NB = NH // 512     # 4 query blocks of 512
NCORES = 8

_cache = {}


def _col(ap_1d, ct):
    # View a [256] DRAM tensor as [256, 1] and take channel-tile ct's [128, 1].
    return ap_1d.ap().rearrange("(a b) -> a b", b=1)[ct * P:(ct + 1) * P, :]


def _build_program():
    nc = bacc.Bacc("TRN2", target_bir_lowering=False, debug=False)

    x_full = nc.dram_tensor("x_full", [C, N], F32, kind="ExternalInput")
    xh = nc.dram_tensor("xh", [C, NH], F32, kind="ExternalInput")
    gnsc = nc.dram_tensor("gnsc", [C], F32, kind="ExternalInput")
    gnbs = nc.dram_tensor("gnbs", [C], F32, kind="ExternalInput")
    g8 = nc.dram_tensor("g8", [P, P // GSIZE], F32, kind="ExternalInput")
    gt01 = nc.dram_tensor("gt01", [P // GSIZE, P], F32, kind="ExternalInput")
    wqT = nc.dram_tensor("wqT", [C, C], BF16, kind="ExternalInput")
    bq = nc.dram_tensor("bq", [C], F32, kind="ExternalInput")
    wkT = nc.dram_tensor("wkT", [C, C], BF16, kind="ExternalInput")
    bk = nc.dram_tensor("bk", [C], F32, kind="ExternalInput")
    wvT = nc.dram_tensor("wvT", [C, C], BF16, kind="ExternalInput")
    wpT = nc.dram_tensor("wpT", [C, C], BF16, kind="ExternalInput")
    bpe = nc.dram_tensor("bpe", [C], F32, kind="ExternalInput")
    out = nc.dram_tensor("out", [C, NH], F32, kind="ExternalOutput")
    rinv_scr = nc.dram_tensor("rinv_scr", [NH], F32)

    with tile.TileContext(nc) as tc:
        _body(tc, x_full, xh, gnsc, gnbs, g8, gt01,
              wqT, bq, wkT, bk, wvT, wpT, bpe, out, rinv_scr)
    nc.compile()
    return nc


def _body(tc, x_full, xh, gnsc, gnbs, g8, gt01,
          wqT, bq, wkT, bk, wvT, wpT, bpe, out, rinv_scr):
    nc = tc.nc
    NG = P // GSIZE  # 16 groups per channel tile

    from contextlib import ExitStack
    with ExitStack() as ctx:
        consts = ctx.enter_context(tc.tile_pool(name="consts", bufs=1))
        px = ctx.enter_context(tc.tile_pool(name="px", bufs=1))
        ph = ctx.enter_context(tc.tile_pool(name="ph", bufs=1))
        pkv = ctx.enter_context(tc.tile_pool(name="pkv", bufs=1))
        pet = ctx.enter_context(tc.tile_pool(name="pet", bufs=2))
        pst = ctx.enter_context(tc.tile_pool(name="pst", bufs=4))
        pout = ctx.enter_context(tc.tile_pool(name="pout", bufs=3))
        ps_sc = ctx.enter_context(tc.tile_pool(name="ps_sc", bufs=3, space="PSUM"))
        ps_sum = ctx.enter_context(tc.tile_pool(name="ps_sum", bufs=2, space="PSUM"))
        ps_pv = ctx.enter_context(tc.tile_pool(name="ps_pv", bufs=3, space="PSUM"))

        # ---- constants ----
        ones_b = consts.tile([P, 1], BF16, tag="ones")
        nc.vector.memset(ones_b, 1.0)
        g8_sb = consts.tile([P, NG], F32, tag="g8")
        nc.sync.dma_start(out=g8_sb, in_=g8.ap())
        gt01_sb = consts.tile([NG, P], F32, tag="gt01")
        nc.sync.dma_start(out=gt01_sb, in_=gt01.ap())

        w_sb = {}
        for name, h in (("wqT", wqT), ("wkT", wkT), ("wvT", wvT), ("wpT", wpT)):
            for ec in range(CT):
                t = consts.tile([P, C], BF16, tag=f"{name}{ec}")
                nc.sync.dma_start(out=t, in_=h.ap()[ec * P:(ec + 1) * P, :])
                w_sb[(name, ec)] = t

        col_sb = {}
        for name, h in (("gnsc", gnsc), ("gnbs", gnbs), ("bq", bq),
                        ("bk", bk), ("bpe", bpe)):
            for ct in range(CT):
                t = consts.tile([P, 1], F32, tag=f"{name}{ct}")
                nc.sync.dma_start(out=t, in_=_col(h, ct))
                col_sb[(name, ct)] = t

        # ---- load x (spread across DMA queues), GroupNorm stats ----
        x_sb = []
        dma_engs = [nc.sync, nc.gpsimd, nc.scalar, nc.sync]
        for ct in range(CT):
            xt = px.tile([P, N], F32, tag=f"x{ct}", name=f"x{ct}")
            for c4 in range(4):
                dma_engs[(ct * 4 + c4) % 4].dma_start(
                    out=xt[:, c4 * 1024:(c4 + 1) * 1024],
                    in_=x_full.ap()[ct * P:(ct + 1) * P, c4 * 1024:(c4 + 1) * 1024])
            x_sb.append(xt)

        ab_cols = []
        for ct in range(CT):
            xt = x_sb[ct]
            stats = pst.tile([P, 8, nc.vector.BN_STATS_DIM], F32, tag="bnst")
            for s in range(8):
                nc.vector.bn_stats(out=stats[:, s, :], in_=xt[:, s * 512:(s + 1) * 512])
            mv = pst.tile([P, nc.vector.BN_AGGR_DIM], F32, tag="bnagg")
            nc.vector.bn_aggr(out=mv, in_=stats)

            # per-channel (mean, E[x^2]) -> per-group via G/8 matmul
            st2 = pst.tile([P, 2], F32, tag="st2")
            nc.vector.tensor_copy(out=st2[:, 0:1], in_=mv[:, 0:1])
            m2 = pst.tile([P, 1], F32, tag="m2")
            nc.vector.tensor_mul(m2, mv[:, 0:1], mv[:, 0:1])
            nc.vector.tensor_add(st2[:, 1:2], m2, mv[:, 1:2])

            gps = ps_sc.tile([NG, 2], F32, tag="sc")
            nc.tensor.matmul(gps, lhsT=g8_sb, rhs=st2, start=True, stop=True)
            gs = pst.tile([NG, 2], F32, tag="gs")
            nc.vector.tensor_copy(out=gs, in_=gps)

            # var_g = E[x^2]_g - mean_g^2 ; rstd = 1/sqrt(var+eps)
            vg = pst.tile([NG, 1], F32, tag="vg")
            nc.vector.tensor_mul(vg, gs[:, 0:1], gs[:, 0:1])
            nc.vector.tensor_tensor(out=vg, in0=gs[:, 1:2], in1=vg,
                                    op=AluOpType.subtract)
            eps_t = pst.tile([NG, 1], F32, tag="eps")
            nc.vector.memset(eps_t, EPS)
            std = pst.tile([NG, 1], F32, tag="std")
            nc.scalar.activation(out=std, in_=vg, func=AF.Sqrt, bias=eps_t, scale=1.0)
            rstd = pst.tile([NG, 1], F32, tag="rstd")
            nc.vector.reciprocal(out=rstd, in_=std)

            gs2 = pst.tile([NG, 2], F32, tag="gs2")
            nc.vector.tensor_copy(out=gs2[:, 0:1], in_=gs[:, 0:1])
            nc.vector.tensor_copy(out=gs2[:, 1:2], in_=rstd)

            # broadcast (mean_g, rstd_g) back to channels
            bps = ps_sc.tile([P, 2], F32, tag="sc")
            nc.tensor.matmul(bps, lhsT=gt01_sb, rhs=gs2, start=True, stop=True)
            mr = pst.tile([P, 2], F32, tag="mr")
            nc.vector.tensor_copy(out=mr, in_=bps)

            a_col = pst.tile([P, 1], F32, tag=f"acol{ct}")
            nc.vector.tensor_mul(a_col, mr[:, 1:2], col_sb[("gnsc", ct)])
            b_col = pst.tile([P, 1], F32, tag=f"bcol{ct}")
            nc.vector.tensor_mul(b_col, mr[:, 0:1], a_col)
            nc.vector.tensor_tensor(out=b_col, in0=col_sb[("gnbs", ct)],
                                    in1=b_col, op=AluOpType.subtract)
            ab_cols.append((a_col, b_col))

        # ---- h = x*A+B (chunked so k/vT matmuls start early), k, vT ----
        h_sb = [ph.tile([P, N], BF16, tag=f"h{ct}", name=f"h{ct}") for ct in range(CT)]
        k_sb = [pkv.tile([P, N], BF16, tag=f"k{dt}", name=f"k{dt}") for dt in range(CT)]
        vT_sb = pkv.tile([P, JT, C], BF16, tag="vT")
        for c4 in range(4):
            j0 = c4 * 1024
            for ct in range(CT):
                a_col, b_col = ab_cols[ct]
                nc.vector.tensor_scalar(
                    out=h_sb[ct][:, j0:j0 + 1024], in0=x_sb[ct][:, j0:j0 + 1024],
                    scalar1=a_col, scalar2=b_col,
                    op0=AluOpType.mult, op1=AluOpType.add)
            for jc in (2 * c4, 2 * c4 + 1):
                for dt in range(CT):
                    ps = ps_sc.tile([P, 512], F32, tag="sc")
                    for ec in range(CT):
                        nc.tensor.matmul(
                            ps, lhsT=w_sb[("wkT", ec)][:, dt * P:(dt + 1) * P],
                            rhs=h_sb[ec][:, jc * 512:(jc + 1) * 512],
                            start=(ec == 0), stop=(ec == CT - 1))
                    nc.scalar.activation(
                        out=k_sb[dt][:, jc * 512:(jc + 1) * 512], in_=ps,
                        func=AF.Identity, bias=col_sb[("bk", dt)], scale=1.0)
            for jt in range(8 * c4, 8 * c4 + 8):
                ps = ps_pv.tile([P, 512], F32, tag="pv")
                for ec in range(CT):
                    nc.tensor.matmul(
                        ps[:, 0:C], lhsT=h_sb[ec][:, jt * P:(jt + 1) * P],
                        rhs=w_sb[("wvT", ec)],
                        start=(ec == 0), stop=(ec == CT - 1))
                nc.vector.tensor_copy(out=vT_sb[:, jt, :], in_=ps[:, 0:C])

        # ---- query-half h, q ----
        xh_sb, hh_sb = [], []
        for ct in range(CT):
            xht = px.tile([P, NH], F32, tag=f"x{ct}", name=f"xh{ct}")
            dma_engs[ct].dma_start(out=xht, in_=xh.ap()[ct * P:(ct + 1) * P, :])
            xh_sb.append(xht)
            a_col, b_col = ab_cols[ct]
            hht = ph.tile([P, NH], BF16, tag=f"hh{ct}", name=f"hh{ct}")
            nc.vector.tensor_scalar(out=hht, in0=xht, scalar1=a_col, scalar2=b_col,
                                    op0=AluOpType.mult, op1=AluOpType.add)
            hh_sb.append(hht)

        q_sb = [pkv.tile([P, NH], BF16, tag=f"q{dt}", name=f"q{dt}") for dt in range(CT)]
        for dt in range(CT):
            for ic in range(NH // 512):
                ps = ps_sc.tile([P, 512], F32, tag="sc")
                for ec in range(CT):
                    nc.tensor.matmul(
                        ps, lhsT=w_sb[("wqT", ec)][:, dt * P:(dt + 1) * P],
                        rhs=hh_sb[ec][:, ic * 512:(ic + 1) * 512],
                        start=(ec == 0), stop=(ec == CT - 1))
                nc.scalar.activation(
                    out=q_sb[dt][:, ic * 512:(ic + 1) * 512], in_=ps,
                    func=AF.Identity, bias=col_sb[("bq", dt)], scale=1.0)

        # ---- attention: i-blocks of 512 processed in pairs so each k/vT
        # stationary operand serves two matmuls (hides LDWEIGHTS) ----
        A_sb = [pkv.tile([P, NH], BF16, tag=f"A{ct}", name=f"A{ct}") for ct in range(CT)]
        rinvb = pkv.tile([P, NH], F32, tag="rinvb")
        for pp in range(NB // 2):
            eTs = [pet.tile([P, JT, 512], BF16, tag="eT", name=f"eT{pp}_{w}")
                   for w in range(2)]
            i0s = [(2 * pp + w) * 512 for w in range(2)]
            # scores + exp
            for jt in range(JT):
                pss = [ps_sc.tile([P, 512], F32, tag="sc", name=f"sc{pp}_{jt}_{w}")
                       for w in range(2)]
                for cc in range(CT):
                    for w in range(2):
                        nc.tensor.matmul(
                            pss[w], lhsT=k_sb[cc][:, jt * P:(jt + 1) * P],
                            rhs=q_sb[cc][:, i0s[w]:i0s[w] + 512],
                            start=(cc == 0), stop=(cc == CT - 1))
                for w in range(2):
                    nc.scalar.activation(out=eTs[w][:, jt, :], in_=pss[w],
                                         func=AF.Exp)
            # row sums (of exp) via ones-vector matmuls; rinv via DRAM-bounce
            # broadcast, reciprocal taken on the broadcast (all partitions busy)
            for w in range(2):
                i0 = i0s[w]
                pss = ps_sum.tile([1, 512], F32, tag="sm", name=f"sm{pp}_{w}")
                for jt in range(JT):
                    nc.tensor.matmul(pss, lhsT=ones_b, rhs=eTs[w][:, jt, :],
                                     start=(jt == 0), stop=(jt == JT - 1))
                srow = pst.tile([1, 512], F32, tag="srow")
                nc.vector.tensor_copy(out=srow, in_=pss)
                nc.sync.dma_start(
                    out=rinv_scr.ap().rearrange("(a b) -> a b", a=1)[:, i0:i0 + 512],
                    in_=srow)
                rsc = rinv_scr.ap()[i0:i0 + 512]
                sb = pout.tile([P, 512], F32, tag="sb")
                nc.gpsimd.dma_start(
                    out=sb,
                    in_=bass.AP(tensor=rsc.tensor, offset=rsc.offset,
                                ap=[[0, P]] + [list(d) for d in rsc.ap]))
                # 1/s as exp(-ln(s)) on ACT (DVE reciprocal is ~8 cyc/elem)
                nc.scalar.activation(out=sb, in_=sb, func=AF.Ln)
                nc.scalar.activation(out=rinvb[:, i0:i0 + 512], in_=sb,
                                     func=AF.Exp, scale=-1.0)
            # PV: ct sequential, vT stationary operand reused across the pair
            for ct in range(CT):
                psas = [ps_pv.tile([P, 512], F32, tag="pv", name=f"pv{pp}_{ct}_{w}")
                        for w in range(2)]
                for jt in range(JT):
                    for w in range(2):
                        nc.tensor.matmul(
                            psas[w], lhsT=vT_sb[:, jt, ct * P:(ct + 1) * P],
                            rhs=eTs[w][:, jt, :],
                            start=(jt == 0), stop=(jt == JT - 1))
                for w in range(2):
                    nc.scalar.activation(out=A_sb[ct][:, i0s[w]:i0s[w] + 512],
                                         in_=psas[w], func=AF.Copy)
            # output projection + normalization + bias + residual for this pair
            for w in range(2):
                ic = 2 * pp + w
                for dt in range(CT):
                    ps = ps_pv.tile([P, 512], F32, tag="pv", name=f"pj{pp}_{w}_{dt}")
                    for cc in range(CT):
                        nc.tensor.matmul(
                            ps, lhsT=w_sb[("wpT", cc)][:, dt * P:(dt + 1) * P],
                            rhs=A_sb[cc][:, ic * 512:(ic + 1) * 512],
                            start=(cc == 0), stop=(cc == CT - 1))
                    ot = pout.tile([P, 512], F32, tag="ot")
                    nc.vector.tensor_mul(ot, ps, rinvb[:, ic * 512:(ic + 1) * 512])
                    nc.vector.tensor_scalar(out=ot, in0=ot,
                                            scalar1=col_sb[("bpe", dt)],
                                            scalar2=None, op0=AluOpType.add)
                    nc.vector.tensor_add(ot, ot, xh_sb[dt][:, ic * 512:(ic + 1) * 512])
                    nc.sync.dma_start(
                        out=out.ap()[dt * P:(dt + 1) * P, ic * 512:(ic + 1) * 512],
                        in_=ot)


def _get_program():
    if "nc" not in _cache:
        _cache["nc"] = _build_program()
    return _cache["nc"]


def kernel(x, gn_scale, gn_bias, wq, bq, wk, bk, wv, bv, wproj, bproj):
    x = np.asarray(x, dtype=np.float32)
    b, c, hh, ww = x.shape
    assert (b, c, hh * ww) == (B, C, N)
    xf = np.ascontiguousarray(x.reshape(B, C, N))

    bf = ml_dtypes.bfloat16
    wqT_s = np.ascontiguousarray((np.asarray(wq, np.float32).T / np.sqrt(C))).astype(bf)
    bq_s = (np.asarray(bq, np.float32) / np.sqrt(C)).astype(np.float32)
    wkT = np.ascontiguousarray(np.asarray(wk, np.float32).T).astype(bf)
    wvT = np.ascontiguousarray(np.asarray(wv, np.float32).T).astype(bf)
    wpT = np.ascontiguousarray(np.asarray(wproj, np.float32).T).astype(bf)
    # softmax rows sum to 1 => v-bias contributes wproj@bv, constant per channel
    bpe = (np.asarray(bproj, np.float64)
           + np.asarray(wproj, np.float64) @ np.asarray(bv, np.float64)
           ).astype(np.float32)

    g8 = np.zeros((P, P // GSIZE), np.float32)
    gt01 = np.zeros((P // GSIZE, P), np.float32)
    for ch in range(P):
        g8[ch, ch // GSIZE] = 1.0 / (GSIZE * N)
        gt01[ch // GSIZE, ch] = 1.0
    # g8 entries 1/(8*4096): the G-matmul then yields per-group means directly
    # (bn per-channel stats are means over 4096, so scale by 4096/(8*4096)=1/8)
    g8 *= N

    common = dict(gnsc=np.asarray(gn_scale, np.float32),
                  gnbs=np.asarray(gn_bias, np.float32),
                  g8=g8, gt01=gt01,
                  wqT=wqT_s, bq=bq_s, wkT=wkT,
                  bk=np.asarray(bk, np.float32),
                  wvT=wvT, wpT=wpT, bpe=bpe)

    in_maps = []
    for core in range(NCORES):
        bi, half = core // 2, core % 2
        in_maps.append(dict(
            x_full=np.ascontiguousarray(xf[bi]),
            xh=np.ascontiguousarray(xf[bi][:, half * NH:(half + 1) * NH]),
            **common))

    nc = _get_program()
    trace = bool(os.environ.get("BASS_KERNEL_TRACE"))
    res = run_bass_kernel_spmd(nc, in_maps, core_ids=list(range(NCORES)),
                               trace=trace)
    _cache["last_results"] = res

    full = np.empty((B, C, N), np.float32)
    for core in range(NCORES):
        bi, half = core // 2, core % 2
        full[bi][:, half * NH:(half + 1) * NH] = res.results[core]["out"]
    return full.reshape(B, C, hh, ww)


# revision 10
# speedup vs baseline: 1.1015x; 1.0348x over previous
# Trainium2 Bass kernel for NonLocalBlock (GroupNorm + 1x1-conv self-attention + residual).
#
# Full input x: [4, 256, 64, 64] f32. Output: x + proj(attn(gn(x))), same shape.
#
# Sharding: 8 cores = 4 batches x 2 query-halves. Attention is independent per
# batch; within a batch, softmax rows (queries) split cleanly across 2 cores.
# Each core redundantly computes GroupNorm + K + V^T for its batch (cheap), and
# computes scores/softmax/PV/proj only for its 2048 queries. No collectives.
#
# Per-core program layout (c = 256 channels as 2 partition-tiles, n = 4096):
#   - GroupNorm stats: bn_stats/bn_aggr per channel, group-combine and
#     broadcast-back via tiny PE matmuls with 0/1 group matrices.
#   - h = x*A + B (bf16), also on the query half (separate input slice, so all
#     access patterns stay static across the SPMD program).
#   - k[c,j] (bf16), q[c,i] (bf16, 1/sqrt(c) folded into wq on host),
#     vT[j,c] (bf16, computed directly in transposed layout).
#   - scores transposed: sT[j,i] = k^T q via PE; exp on ACT fused with the
#     PSUM->SBUF copy; eT[j,i] is then directly the PV moving operand.
#   - row sums of exp via ones-vector PE matmuls; softmax normalization is a
#     column scaling that commutes through PV and proj, applied at the end.
#   - bv never applied on-chip: softmax rows sum to 1, so wproj@bv folds into
#     bproj on the host.
#   - out = x_half + rinv * (wproj @ A_unnorm) + bproj_eff.

import os
import sys

for _p in ("/opt/trn_rl_repo", "/root/.axon_site/_ro/trn_rl_repo"):
    if os.path.isdir(_p) and _p not in sys.path:
        sys.path.insert(0, _p)

import numpy as np
import ml_dtypes

import concourse.bass as bass
import concourse.tile as tile
from concourse import bacc, mybir
from concourse.alu_op_type import AluOpType
from concourse.bass_utils import run_bass_kernel_spmd

F32 = mybir.dt.float32
BF16 = mybir.dt.bfloat16
AF = mybir.ActivationFunctionType

B = 4
C = 256
N = 4096           # 64*64 spatial positions
NH = N // 2        # queries per core
GROUPS = 32
GSIZE = C // GROUPS  # 8 channels per group
EPS = 1e-6
P = 128
CT = C // P        # 2 channel tiles
JT = N // P        # 32 key tiles
NB = NH // 512     # 4 query blocks of 512
NCORES = 8

_cache = {}


def _col(ap_1d, ct):
    # View a [256] DRAM tensor as [256, 1] and take channel-tile ct's [128, 1].
    return ap_1d.ap().rearrange("(a b) -> a b", b=1)[ct * P:(ct + 1) * P, :]


def _build_program():
    nc = bacc.Bacc("TRN2", target_bir_lowering=False, debug=False)

    x_full = nc.dram_tensor("x_full", [C, N], F32, kind="ExternalInput")
    xh = nc.dram_tensor("xh", [C, NH], F32, kind="ExternalInput")
    gnsc = nc.dram_tensor("gnsc", [C], F32, kind="ExternalInput")
    gnbs = nc.dram_tensor("gnbs", [C], F32, kind="ExternalInput")
    g8 = nc.dram_tensor("g8", [P, P // GSIZE], F32, kind="ExternalInput")
    gt01 = nc.dram_tensor("gt01", [P // GSIZE, P], F32, kind="ExternalInput")
    wqT = nc.dram_tensor("wqT", [C, C], BF16, kind="ExternalInput")
    bq = nc.dram_tensor("bq", [C], F32, kind="ExternalInput")
    wkT = nc.dram_tensor("wkT", [C, C], BF16, kind="ExternalInput")
    bk = nc.dram_tensor("bk", [C], F32, kind="ExternalInput")
    wvT = nc.dram_tensor("wvT", [C, C], BF16, kind="ExternalInput")
    wpT = nc.dram_tensor("wpT", [C, C], BF16, kind="ExternalInput")
    bpe = nc.dram_tensor("bpe", [C], F32, kind="ExternalInput")
    out = nc.dram_tensor("out", [C, NH], F32, kind="ExternalOutput")
    rinv_scr = nc.dram_tensor("rinv_scr", [NH], F32)

    with tile.TileContext(nc) as tc:
        _body(tc, x_full, xh, gnsc, gnbs, g8, gt01,
              wqT, bq, wkT, bk, wvT, wpT, bpe, out, rinv_scr)
    nc.compile()
    return nc


def _body(tc, x_full, xh, gnsc, gnbs, g8, gt01,
          wqT, bq, wkT, bk, wvT, wpT, bpe, out, rinv_scr):
    nc = tc.nc
    NG = P // GSIZE  # 16 groups per channel tile

    from contextlib import ExitStack
    with ExitStack() as ctx:
        consts = ctx.enter_context(tc.tile_pool(name="consts", bufs=1))
        px = ctx.enter_context(tc.tile_pool(name="px", bufs=1))
        ph = ctx.enter_context(tc.tile_pool(name="ph", bufs=1))
        pkv = ctx.enter_context(tc.tile_pool(name="pkv", bufs=1))
        pet = ctx.enter_context(tc.tile_pool(name="pet", bufs=2))
        pst = ctx.enter_context(tc.tile_pool(name="pst", bufs=4))
        pout = ctx.enter_context(tc.tile_pool(name="pout", bufs=3))
        ps_sc = ctx.enter_context(tc.tile_pool(name="ps_sc", bufs=3, space="PSUM"))
        ps_sum = ctx.enter_context(tc.tile_pool(name="ps_sum", bufs=2, space="PSUM"))
        ps_pv = ctx.enter_context(tc.tile_pool(name="ps_pv", bufs=3, space="PSUM"))

        # ---- x load first: one 1MB DMA per (ct, half), two queues ----
        x_sb = []
        for ct in range(CT):
            xt = px.tile([P, N], F32, tag=f"x{ct}", name=f"x{ct}")
            for c2 in range(2):
                [nc.sync, nc.scalar][ct].dma_start(
                    out=xt[:, c2 * 2048:(c2 + 1) * 2048],
                    in_=x_full.ap()[ct * P:(ct + 1) * P, c2 * 2048:(c2 + 1) * 2048])
            x_sb.append(xt)

        # ---- constants (gpsimd queue; keeps x queues clear) ----
        ones_b = consts.tile([P, 1], BF16, tag="ones")
        nc.vector.memset(ones_b, 1.0)
        g8_sb = consts.tile([P, NG], F32, tag="g8")
        nc.gpsimd.dma_start(out=g8_sb, in_=g8.ap())
        gt01_sb = consts.tile([NG, P], F32, tag="gt01")
        nc.gpsimd.dma_start(out=gt01_sb, in_=gt01.ap())

        w_sb = {}
        for name, h in (("wqT", wqT), ("wkT", wkT), ("wvT", wvT), ("wpT", wpT)):
            for ec in range(CT):
                t = consts.tile([P, C], BF16, tag=f"{name}{ec}")
                nc.gpsimd.dma_start(out=t, in_=h.ap()[ec * P:(ec + 1) * P, :])
                w_sb[(name, ec)] = t

        col_sb = {}
        for name, h in (("gnsc", gnsc), ("gnbs", gnbs), ("bq", bq),
                        ("bk", bk), ("bpe", bpe)):
            for ct in range(CT):
                t = consts.tile([P, 1], F32, tag=f"{name}{ct}")
                nc.gpsimd.dma_start(out=t, in_=_col(h, ct))
                col_sb[(name, ct)] = t

        # ---- GroupNorm stats ----
        ab_cols = []
        for ct in range(CT):
            xt = x_sb[ct]
            stats = pst.tile([P, 8, nc.vector.BN_STATS_DIM], F32, tag="bnst")
            for s in range(8):
                nc.vector.bn_stats(out=stats[:, s, :], in_=xt[:, s * 512:(s + 1) * 512])
            mv = pst.tile([P, nc.vector.BN_AGGR_DIM], F32, tag="bnagg")
            nc.vector.bn_aggr(out=mv, in_=stats)

            # per-channel (mean, E[x^2]) -> per-group via G/8 matmul
            st2 = pst.tile([P, 2], F32, tag="st2")
            nc.vector.tensor_copy(out=st2[:, 0:1], in_=mv[:, 0:1])
            m2 = pst.tile([P, 1], F32, tag="m2")
            nc.vector.tensor_mul(m2, mv[:, 0:1], mv[:, 0:1])
            nc.vector.tensor_add(st2[:, 1:2], m2, mv[:, 1:2])

            gps = ps_sc.tile([NG, 2], F32, tag="sc")
            nc.tensor.matmul(gps, lhsT=g8_sb, rhs=st2, start=True, stop=True)
            gs = pst.tile([NG, 2], F32, tag="gs")
            nc.vector.tensor_copy(out=gs, in_=gps)

            # var_g = E[x^2]_g - mean_g^2 ; rstd = 1/sqrt(var+eps)
            vg = pst.tile([NG, 1], F32, tag="vg")
            nc.vector.tensor_mul(vg, gs[:, 0:1], gs[:, 0:1])
            nc.vector.tensor_tensor(out=vg, in0=gs[:, 1:2], in1=vg,
                                    op=AluOpType.subtract)
            eps_t = pst.tile([NG, 1], F32, tag="eps")
            nc.vector.memset(eps_t, EPS)
            std = pst.tile([NG, 1], F32, tag="std")
            nc.scalar.activation(out=std, in_=vg, func=AF.Sqrt, bias=eps_t, scale=1.0)
            rstd = pst.tile([NG, 1], F32, tag="rstd")
            nc.vector.reciprocal(out=rstd, in_=std)

            gs2 = pst.tile([NG, 2], F32, tag="gs2")
            nc.vector.tensor_copy(out=gs2[:, 0:1], in_=gs[:, 0:1])
            nc.vector.tensor_copy(out=gs2[:, 1:2], in_=rstd)

            # broadcast (mean_g, rstd_g) back to channels
            bps = ps_sc.tile([P, 2], F32, tag="sc")
            nc.tensor.matmul(bps, lhsT=gt01_sb, rhs=gs2, start=True, stop=True)
            mr = pst.tile([P, 2], F32, tag="mr")
            nc.vector.tensor_copy(out=mr, in_=bps)

            a_col = pst.tile([P, 1], F32, tag=f"acol{ct}")
            nc.vector.tensor_mul(a_col, mr[:, 1:2], col_sb[("gnsc", ct)])
            b_col = pst.tile([P, 1], F32, tag=f"bcol{ct}")
            nc.vector.tensor_mul(b_col, mr[:, 0:1], a_col)
            nc.vector.tensor_tensor(out=b_col, in0=col_sb[("gnbs", ct)],
                                    in1=b_col, op=AluOpType.subtract)
            ab_cols.append((a_col, b_col))

        # ---- h = x*A+B (chunked so k/vT matmuls start early), k, vT ----
        h_sb = [ph.tile([P, N], BF16, tag=f"h{ct}", name=f"h{ct}") for ct in range(CT)]
        k_sb = [pkv.tile([P, N], BF16, tag=f"k{dt}", name=f"k{dt}") for dt in range(CT)]
        vT_sb = pkv.tile([P, JT, C], BF16, tag="vT")
        for c4 in range(4):
            j0 = c4 * 1024
            for ct in range(CT):
                a_col, b_col = ab_cols[ct]
                nc.vector.tensor_scalar(
                    out=h_sb[ct][:, j0:j0 + 1024], in0=x_sb[ct][:, j0:j0 + 1024],
                    scalar1=a_col, scalar2=b_col,
                    op0=AluOpType.mult, op1=AluOpType.add)
            for jc in (2 * c4, 2 * c4 + 1):
                for dt in range(CT):
                    ps = ps_sc.tile([P, 512], F32, tag="sc")
                    for ec in range(CT):
                        nc.tensor.matmul(
                            ps, lhsT=w_sb[("wkT", ec)][:, dt * P:(dt + 1) * P],
                            rhs=h_sb[ec][:, jc * 512:(jc + 1) * 512],
                            start=(ec == 0), stop=(ec == CT - 1))
                    nc.scalar.activation(
                        out=k_sb[dt][:, jc * 512:(jc + 1) * 512], in_=ps,
                        func=AF.Identity, bias=col_sb[("bk", dt)], scale=1.0)
            for jt in range(8 * c4, 8 * c4 + 8):
                ps = ps_pv.tile([P, 512], F32, tag="pv")
                for ec in range(CT):
                    nc.tensor.matmul(
                        ps[:, 0:C], lhsT=h_sb[ec][:, jt * P:(jt + 1) * P],
                        rhs=w_sb[("wvT", ec)],
                        start=(ec == 0), stop=(ec == CT - 1))
                nc.vector.tensor_copy(out=vT_sb[:, jt, :], in_=ps[:, 0:C])

        # ---- query-half h, q ----
        xh_sb, hh_sb = [], []
        for ct in range(CT):
            xht = px.tile([P, NH], F32, tag=f"x{ct}", name=f"xh{ct}")
            [nc.sync, nc.scalar][ct].dma_start(out=xht, in_=xh.ap()[ct * P:(ct + 1) * P, :])
            xh_sb.append(xht)
            a_col, b_col = ab_cols[ct]
            hht = ph.tile([P, NH], BF16, tag=f"hh{ct}", name=f"hh{ct}")
            nc.vector.tensor_scalar(out=hht, in0=xht, scalar1=a_col, scalar2=b_col,
                                    op0=AluOpType.mult, op1=AluOpType.add)
            hh_sb.append(hht)

        q_sb = [pkv.tile([P, NH], BF16, tag=f"q{dt}", name=f"q{dt}") for dt in range(CT)]
        for dt in range(CT):
            for ic in range(NH // 512):
                ps = ps_sc.tile([P, 512], F32, tag="sc")
                for ec in range(CT):
                    nc.tensor.matmul(
                        ps, lhsT=w_sb[("wqT", ec)][:, dt * P:(dt + 1) * P],
                        rhs=hh_sb[ec][:, ic * 512:(ic + 1) * 512],
                        start=(ec == 0), stop=(ec == CT - 1))
                nc.scalar.activation(
                    out=q_sb[dt][:, ic * 512:(ic + 1) * 512], in_=ps,
                    func=AF.Identity, bias=col_sb[("bq", dt)], scale=1.0)

        # ---- attention: i-blocks of 512 processed in pairs so each k/vT
        # stationary operand serves two matmuls (hides LDWEIGHTS) ----
        A_sb = [pkv.tile([P, NH], BF16, tag=f"A{ct}", name=f"A{ct}") for ct in range(CT)]
        rinvb = pkv.tile([P, NH], F32, tag="rinvb")
        for pp in range(NB // 2):
            eTs = [pet.tile([P, JT, 512], BF16, tag="eT", name=f"eT{pp}_{w}")
                   for w in range(2)]
            i0s = [(2 * pp + w) * 512 for w in range(2)]
            # scores + exp
            for jt in range(JT):
                pss = [ps_sc.tile([P, 512], F32, tag="sc", name=f"sc{pp}_{jt}_{w}")
                       for w in range(2)]
                for cc in range(CT):
                    for w in range(2):
                        nc.tensor.matmul(
                            pss[w], lhsT=k_sb[cc][:, jt * P:(jt + 1) * P],
                            rhs=q_sb[cc][:, i0s[w]:i0s[w] + 512],
                            start=(cc == 0), stop=(cc == CT - 1))
                for w in range(2):
                    nc.scalar.activation(out=eTs[w][:, jt, :], in_=pss[w],
                                         func=AF.Exp)
            # row sums (of exp) via ones-vector matmuls; rinv via DRAM-bounce
            # broadcast, reciprocal taken on the broadcast (all partitions busy)
            for w in range(2):
                i0 = i0s[w]
                pss = ps_sum.tile([1, 512], F32, tag="sm", name=f"sm{pp}_{w}")
                for jt in range(JT):
                    nc.tensor.matmul(pss, lhsT=ones_b, rhs=eTs[w][:, jt, :],
                                     start=(jt == 0), stop=(jt == JT - 1))
                srow = pst.tile([1, 512], F32, tag="srow")
                nc.vector.tensor_copy(out=srow, in_=pss)
                nc.sync.dma_start(
                    out=rinv_scr.ap().rearrange("(a b) -> a b", a=1)[:, i0:i0 + 512],
                    in_=srow)
                rsc = rinv_scr.ap()[i0:i0 + 512]
                sb = pout.tile([P, 512], F32, tag="sb")
                nc.gpsimd.dma_start(
                    out=sb,
                    in_=bass.AP(tensor=rsc.tensor, offset=rsc.offset,
                                ap=[[0, P]] + [list(d) for d in rsc.ap]))
                # ~18-bit reciprocal, single DVE op (plenty for softmax scale)
                nc.vector.reciprocal_approx_fast(out=rinvb[:, i0:i0 + 512], in_=sb)
            # PV: ct sequential, vT stationary operand reused across the pair
            for ct in range(CT):
                psas = [ps_pv.tile([P, 512], F32, tag="pv", name=f"pv{pp}_{ct}_{w}")
                        for w in range(2)]
                for jt in range(JT):
                    for w in range(2):
                        nc.tensor.matmul(
                            psas[w], lhsT=vT_sb[:, jt, ct * P:(ct + 1) * P],
                            rhs=eTs[w][:, jt, :],
                            start=(jt == 0), stop=(jt == JT - 1))
                for w in range(2):
                    nc.scalar.activation(out=A_sb[ct][:, i0s[w]:i0s[w] + 512],
                                         in_=psas[w], func=AF.Copy)
            # output projection + normalization + bias + residual for this pair
            for w in range(2):
                ic = 2 * pp + w
                for dt in range(CT):
                    ps = ps_pv.tile([P, 512], F32, tag="pv", name=f"pj{pp}_{w}_{dt}")
                    for cc in range(CT):
                        nc.tensor.matmul(
                            ps, lhsT=w_sb[("wpT", cc)][:, dt * P:(dt + 1) * P],
                            rhs=A_sb[cc][:, ic * 512:(ic + 1) * 512],
                            start=(cc == 0), stop=(cc == CT - 1))
                    ot = pout.tile([P, 512], F32, tag="ot")
                    nc.vector.tensor_mul(ot, ps, rinvb[:, ic * 512:(ic + 1) * 512])
                    nc.vector.tensor_scalar(out=ot, in0=ot,
                                            scalar1=col_sb[("bpe", dt)],
                                            scalar2=None, op0=AluOpType.add)
                    nc.vector.tensor_add(ot, ot, xh_sb[dt][:, ic * 512:(ic + 1) * 512])
                    nc.sync.dma_start(
                        out=out.ap()[dt * P:(dt + 1) * P, ic * 512:(ic + 1) * 512],
                        in_=ot)


def _get_program():
    if "nc" not in _cache:
        _cache["nc"] = _build_program()
    return _cache["nc"]


def kernel(x, gn_scale, gn_bias, wq, bq, wk, bk, wv, bv, wproj, bproj):
    x = np.asarray(x, dtype=np.float32)
    b, c, hh, ww = x.shape
    assert (b, c, hh * ww) == (B, C, N)
    xf = np.ascontiguousarray(x.reshape(B, C, N))

    bf = ml_dtypes.bfloat16
    wqT_s = np.ascontiguousarray((np.asarray(wq, np.float32).T / np.sqrt(C))).astype(bf)
    bq_s = (np.asarray(bq, np.float32) / np.sqrt(C)).astype(np.float32)
    wkT = np.ascontiguousarray(np.asarray(wk, np.float32).T).astype(bf)
    wvT = np.ascontiguousarray(np.asarray(wv, np.float32).T).astype(bf)
    wpT = np.ascontiguousarray(np.asarray(wproj, np.float32).T).astype(bf)
    # softmax rows sum to 1 => v-bias contributes wproj@bv, constant per channel
    bpe = (np.asarray(bproj, np.float64)
           + np.asarray(wproj, np.float64) @ np.asarray(bv, np.float64)
           ).astype(np.float32)

    g8 = np.zeros((P, P // GSIZE), np.float32)
    gt01 = np.zeros((P // GSIZE, P), np.float32)
    for ch in range(P):
        g8[ch, ch // GSIZE] = 1.0 / (GSIZE * N)
        gt01[ch // GSIZE, ch] = 1.0
    # g8 entries 1/(8*4096): the G-matmul then yields per-group means directly
    # (bn per-channel stats are means over 4096, so scale by 4096/(8*4096)=1/8)
    g8 *= N

    common = dict(gnsc=np.asarray(gn_scale, np.float32),
                  gnbs=np.asarray(gn_bias, np.float32),
                  g8=g8, gt01=gt01,
                  wqT=wqT_s, bq=bq_s, wkT=wkT,
                  bk=np.asarray(bk, np.float32),
                  wvT=wvT, wpT=wpT, bpe=bpe)

    in_maps = []
    for core in range(NCORES):
        bi, half = core // 2, core % 2
        in_maps.append(dict(
            x_full=np.ascontiguousarray(xf[bi]),
            xh=np.ascontiguousarray(xf[bi][:, half * NH:(half + 1) * NH]),
            **common))

    nc = _get_program()
    trace = bool(os.environ.get("BASS_KERNEL_TRACE"))
    res = run_bass_kernel_spmd(nc, in_maps, core_ids=list(range(NCORES)),
                               trace=trace)
    _cache["last_results"] = res

    full = np.empty((B, C, N), np.float32)
    for core in range(NCORES):
        bi, half = core // 2, core % 2
        full[bi][:, half * NH:(half + 1) * NH] = res.results[core]["out"]
    return full.reshape(B, C, hh, ww)


# revision 13
# speedup vs baseline: 1.4433x; 1.3103x over previous
# Trainium2 Bass kernel for NonLocalBlock (GroupNorm + 1x1-conv self-attention + residual).
#
# Full input x: [4, 256, 64, 64] f32. Output: x + proj(attn(gn(x))), same shape.
#
# Sharding: 8 cores = 4 batches x 2 query-halves. Attention is independent per
# batch; within a batch, softmax rows (queries) split cleanly across 2 cores.
# Each core redundantly computes GroupNorm + K + V^T for its batch (cheap), and
# computes scores/softmax/PV/proj only for its 2048 queries. No collectives.
#
# Per-core program (c = 256 channels as 2 partition-tiles, n = 4096 keys):
#   - GroupNorm stats: bn_stats/bn_aggr per channel, group-combine and
#     broadcast-back via tiny PE matmuls with 0/1 group matrices.
#   - h = x*A + B (bf16), plus the query half from a separate input slice so
#     all access patterns stay static across the SPMD program.
#   - k, q, vT in fp8-e4m3 with the contraction dim stored channel-interleaved
#     ([128, 2, *]), so the attention matmuls run in DoubleRow perf mode
#     (2 fp8 MACs/cell/cycle, K=256 per instruction). The interleave is
#     produced for free: host permutes weight columns; PSUM->SBUF copies land
#     each output-channel half in its pair plane.
#   - scores transposed: sT[j,i] = k^T q; exp on ACT fused with the
#     PSUM->SBUF copy (1/sqrt(c) folded into the activation scale); eT[j,i]
#     is then directly the PV moving operand - no transposes anywhere.
#   - row sums of exp via ones-vector DR matmuls; softmax normalization is a
#     column scaling that commutes through PV and proj, applied in the output
#     stage (reciprocal_approx_fast on a broadcast of the sums).
#   - bv never applied on-chip: softmax rows sum to 1, so wproj@bv folds into
#     bproj on the host. out = x_half + rinv * (wproj @ A_unnorm) + bproj_eff.
#
# Stationary-operand reuse: each k/vT slice serves all 4 query blocks
# back-to-back, so LDWEIGHTS is paid once per 4 matmuls.

import os
import sys

for _p in ("/opt/trn_rl_repo", "/root/.axon_site/_ro/trn_rl_repo"):
    if os.path.isdir(_p) and _p not in sys.path:
        sys.path.insert(0, _p)

import numpy as np
import ml_dtypes

import concourse.bass as bass
import concourse.tile as tile
from concourse import bacc, mybir
from concourse.alu_op_type import AluOpType
from concourse.bass_utils import run_bass_kernel_spmd

F32 = mybir.dt.float32
BF16 = mybir.dt.bfloat16
F8 = mybir.dt.float8e4
AF = mybir.ActivationFunctionType
DR = mybir.MatmulPerfMode.DoubleRow

B = 4
C = 256
N = 4096           # 64*64 spatial positions
NH = N // 2        # queries per core
GROUPS = 32
GSIZE = C // GROUPS  # 8 channels per group
EPS = 1e-6
P = 128
CT = C // P        # 2 channel tiles
JT = N // P        # 32 key tiles (16 DoubleRow pairs)
JP = JT // 2
NB = NH // 512     # 4 query blocks of 512
NCORES = 8
SCALE = float(1.0 / np.sqrt(C))

_cache = {}


def _col(ap_1d, ct):
    # View a [256] DRAM tensor as [256, 1] and take channel-tile ct's [128, 1].
    return ap_1d.ap().rearrange("(a b) -> a b", b=1)[ct * P:(ct + 1) * P, :]


def _build_program():
    nc = bacc.Bacc("TRN2", target_bir_lowering=False, debug=False)

    x_full = nc.dram_tensor("x_full", [C, N], F32, kind="ExternalInput")
    xh = nc.dram_tensor("xh", [C, NH], F32, kind="ExternalInput")
    gnsc = nc.dram_tensor("gnsc", [C], F32, kind="ExternalInput")
    gnbs = nc.dram_tensor("gnbs", [C], F32, kind="ExternalInput")
    g8 = nc.dram_tensor("g8", [P, P // GSIZE], F32, kind="ExternalInput")
    gt01 = nc.dram_tensor("gt01", [P // GSIZE, P], F32, kind="ExternalInput")
    wqT = nc.dram_tensor("wqT", [C, C], BF16, kind="ExternalInput")
    bq = nc.dram_tensor("bq", [C], F32, kind="ExternalInput")
    wkT = nc.dram_tensor("wkT", [C, C], BF16, kind="ExternalInput")
    bk = nc.dram_tensor("bk", [C], F32, kind="ExternalInput")
    wvT = nc.dram_tensor("wvT", [C, C], BF16, kind="ExternalInput")
    wpT = nc.dram_tensor("wpT", [C, C], BF16, kind="ExternalInput")
    bpe = nc.dram_tensor("bpe", [C], F32, kind="ExternalInput")
    out = nc.dram_tensor("out", [C, NH], F32, kind="ExternalOutput")
    rinv_scr = nc.dram_tensor("rinv_scr", [NH], F32)

    with tile.TileContext(nc) as tc:
        _body(tc, x_full, xh, gnsc, gnbs, g8, gt01,
              wqT, bq, wkT, bk, wvT, wpT, bpe, out, rinv_scr)
    nc.compile()
    return nc


def _body(tc, x_full, xh, gnsc, gnbs, g8, gt01,
          wqT, bq, wkT, bk, wvT, wpT, bpe, out, rinv_scr):
    nc = tc.nc
    NG = P // GSIZE  # 16 groups per channel tile

    from contextlib import ExitStack
    with ExitStack() as ctx:
        consts = ctx.enter_context(tc.tile_pool(name="consts", bufs=1))
        px = ctx.enter_context(tc.tile_pool(name="px", bufs=1))
        ph = ctx.enter_context(tc.tile_pool(name="ph", bufs=1))
        pkv = ctx.enter_context(tc.tile_pool(name="pkv", bufs=1))
        pst = ctx.enter_context(tc.tile_pool(name="pst", bufs=4))
        pout = ctx.enter_context(tc.tile_pool(name="pout", bufs=3))
        # PSUM: three 2-bank "big" slots + two 1-bank sum slots = 8 banks
        ps_big = ctx.enter_context(tc.tile_pool(name="ps_big", bufs=3, space="PSUM"))
        ps_sum = ctx.enter_context(tc.tile_pool(name="ps_sum", bufs=2, space="PSUM"))

        # ---- x load first: one 1MB DMA per (ct, half), two queues ----
        x_sb = []
        for ct in range(CT):
            xt = px.tile([P, N], F32, tag=f"x{ct}", name=f"x{ct}")
            for c2 in range(2):
                [nc.sync, nc.scalar][ct].dma_start(
                    out=xt[:, c2 * 2048:(c2 + 1) * 2048],
                    in_=x_full.ap()[ct * P:(ct + 1) * P, c2 * 2048:(c2 + 1) * 2048])
            x_sb.append(xt)

        # ---- constants (gpsimd queue; keeps x queues clear) ----
        # DR weights need 16B-aligned pair-plane step; pad the ones vector
        ones8_t = consts.tile([P, 2, 16], F8, tag="ones")
        nc.vector.memset(ones8_t, 1.0)
        ones8 = ones8_t[:, :, 0:1]
        g8_sb = consts.tile([P, NG], F32, tag="g8")
        nc.gpsimd.dma_start(out=g8_sb, in_=g8.ap())
        gt01_sb = consts.tile([NG, P], F32, tag="gt01")
        nc.gpsimd.dma_start(out=gt01_sb, in_=gt01.ap())

        w_sb = {}
        for name, h in (("wqT", wqT), ("wkT", wkT), ("wvT", wvT), ("wpT", wpT)):
            for ec in range(CT):
                t = consts.tile([P, C], BF16, tag=f"{name}{ec}")
                nc.gpsimd.dma_start(out=t, in_=h.ap()[ec * P:(ec + 1) * P, :])
                w_sb[(name, ec)] = t

        col_sb = {}
        for name, h in (("gnsc", gnsc), ("gnbs", gnbs), ("bq", bq),
                        ("bk", bk), ("bpe", bpe)):
            for ct in range(CT):
                t = consts.tile([P, 1], F32, tag=f"{name}{ct}")
                nc.gpsimd.dma_start(out=t, in_=_col(h, ct))
                col_sb[(name, ct)] = t

        # ---- GroupNorm stats ----
        ab_cols = []
        for ct in range(CT):
            xt = x_sb[ct]
            stats = pst.tile([P, 8, nc.vector.BN_STATS_DIM], F32, tag="bnst")
            for s in range(8):
                nc.vector.bn_stats(out=stats[:, s, :], in_=xt[:, s * 512:(s + 1) * 512])
            mv = pst.tile([P, nc.vector.BN_AGGR_DIM], F32, tag="bnagg")
            nc.vector.bn_aggr(out=mv, in_=stats)

            # per-channel (mean, E[x^2]) -> per-group via G/8 matmul
            st2 = pst.tile([P, 2], F32, tag="st2")
            nc.vector.tensor_copy(out=st2[:, 0:1], in_=mv[:, 0:1])
            m2 = pst.tile([P, 1], F32, tag="m2")
            nc.vector.tensor_mul(m2, mv[:, 0:1], mv[:, 0:1])
            nc.vector.tensor_add(st2[:, 1:2], m2, mv[:, 1:2])

            gps = ps_big.tile([NG, 2], F32, tag="big")
            nc.tensor.matmul(gps, lhsT=g8_sb, rhs=st2, start=True, stop=True)
            gs = pst.tile([NG, 2], F32, tag="gs")
            nc.vector.tensor_copy(out=gs, in_=gps)

            # var_g = E[x^2]_g - mean_g^2 ; rstd = 1/sqrt(var+eps)
            vg = pst.tile([NG, 1], F32, tag="vg")
            nc.vector.tensor_mul(vg, gs[:, 0:1], gs[:, 0:1])
            nc.vector.tensor_tensor(out=vg, in0=gs[:, 1:2], in1=vg,
                                    op=AluOpType.subtract)
            eps_t = pst.tile([NG, 1], F32, tag="eps")
            nc.vector.memset(eps_t, EPS)
            std = pst.tile([NG, 1], F32, tag="std")
            nc.scalar.activation(out=std, in_=vg, func=AF.Sqrt, bias=eps_t, scale=1.0)
            rstd = pst.tile([NG, 1], F32, tag="rstd")
            nc.vector.reciprocal(out=rstd, in_=std)

            gs2 = pst.tile([NG, 2], F32, tag="gs2")
            nc.vector.tensor_copy(out=gs2[:, 0:1], in_=gs[:, 0:1])
            nc.vector.tensor_copy(out=gs2[:, 1:2], in_=rstd)

            # broadcast (mean_g, rstd_g) back to channels
            bps = ps_big.tile([P, 2], F32, tag="big")
            nc.tensor.matmul(bps, lhsT=gt01_sb, rhs=gs2, start=True, stop=True)
            mr = pst.tile([P, 2], F32, tag="mr")
            nc.vector.tensor_copy(out=mr, in_=bps)

            a_col = pst.tile([P, 1], F32, tag=f"acol{ct}")
            nc.vector.tensor_mul(a_col, mr[:, 1:2], col_sb[("gnsc", ct)])
            b_col = pst.tile([P, 1], F32, tag=f"bcol{ct}")
            nc.vector.tensor_mul(b_col, mr[:, 0:1], a_col)
            nc.vector.tensor_tensor(out=b_col, in0=col_sb[("gnbs", ct)],
                                    in1=b_col, op=AluOpType.subtract)
            ab_cols.append((a_col, b_col))

        # ---- h = x*A+B (chunked so k/vT matmuls start early); k, vT ----
        # k_sb/q_sb/vT_dr are fp8 with channels pair-interleaved for DoubleRow:
        # value (p, q, .) = channel 2p+q (host permuted the weight columns).
        h_sb = [ph.tile([P, N], BF16, tag=f"h{ct}", name=f"h{ct}") for ct in range(CT)]
        k_sb = pkv.tile([P, 2, N], F8, tag="k")
        vT_dr = pkv.tile([P, 2, JP, C], F8, tag="vT")
        for c4 in range(4):
            j0 = c4 * 1024
            for ct in range(CT):
                a_col, b_col = ab_cols[ct]
                nc.vector.tensor_scalar(
                    out=h_sb[ct][:, j0:j0 + 1024], in0=x_sb[ct][:, j0:j0 + 1024],
                    scalar1=a_col, scalar2=b_col,
                    op0=AluOpType.mult, op1=AluOpType.add)
            for dt in range(CT):
                ps = ps_big.tile([P, 1024], F32, tag="big", name=f"k{c4}_{dt}")
                for jj in range(2):
                    jc = 2 * c4 + jj
                    for ec in range(CT):
                        nc.tensor.matmul(
                            ps[:, jj * 512:(jj + 1) * 512],
                            lhsT=w_sb[("wkT", ec)][:, dt * P:(dt + 1) * P],
                            rhs=h_sb[ec][:, jc * 512:(jc + 1) * 512],
                            start=(ec == 0), stop=(ec == CT - 1))
                nc.vector.tensor_scalar(
                    out=k_sb[:, dt, j0:j0 + 1024], in0=ps,
                    scalar1=col_sb[("bk", dt)], scalar2=None, op0=AluOpType.add)
            for jt in range(8 * c4, 8 * c4 + 8):
                ps = ps_big.tile([P, 1024], F32, tag="big", name=f"v{jt}")
                for ec in range(CT):
                    nc.tensor.matmul(
                        ps[:, 0:C], lhsT=h_sb[ec][:, jt * P:(jt + 1) * P],
                        rhs=w_sb[("wvT", ec)],
                        start=(ec == 0), stop=(ec == CT - 1))
                nc.vector.tensor_copy(out=vT_dr[:, jt % 2, jt // 2, :],
                                      in_=ps[:, 0:C])

        # ---- query-half h, q ----
        xh_sb, hh_sb = [], []
        for ct in range(CT):
            xht = px.tile([P, NH], F32, tag=f"x{ct}", name=f"xh{ct}")
            [nc.sync, nc.scalar][ct].dma_start(
                out=xht, in_=xh.ap()[ct * P:(ct + 1) * P, :])
            xh_sb.append(xht)
            a_col, b_col = ab_cols[ct]
            hht = ph.tile([P, NH], BF16, tag=f"hh{ct}", name=f"hh{ct}")
            nc.vector.tensor_scalar(out=hht, in0=xht, scalar1=a_col, scalar2=b_col,
                                    op0=AluOpType.mult, op1=AluOpType.add)
            hh_sb.append(hht)

        q_sb = pkv.tile([P, 2, NH], F8, tag="q")
        for dt in range(CT):
            for icp in range(2):
                ps = ps_big.tile([P, 1024], F32, tag="big", name=f"q{dt}_{icp}")
                for ii in range(2):
                    ic = 2 * icp + ii
                    for ec in range(CT):
                        nc.tensor.matmul(
                            ps[:, ii * 512:(ii + 1) * 512],
                            lhsT=w_sb[("wqT", ec)][:, dt * P:(dt + 1) * P],
                            rhs=hh_sb[ec][:, ic * 512:(ic + 1) * 512],
                            start=(ec == 0), stop=(ec == CT - 1))
                nc.vector.tensor_scalar(
                    out=q_sb[:, dt, icp * 1024:(icp + 1) * 1024], in0=ps,
                    scalar1=col_sb[("bq", dt)], scalar2=None, op0=AluOpType.add)

        # ---- attention: all 2048 queries in one pass over the 32 key tiles.
        # eT[p, jtp, q, i] = exp(s[j=(2*jtp+q)*128+p, i] / 16)  (fp8) ----
        eT = pkv.tile([P, JP, 2, NH], F8, tag="eT")
        A_sb = [pkv.tile([P, NH], BF16, tag=f"A{ct}", name=f"A{ct}")
                for ct in range(CT)]
        rinvb = pkv.tile([P, NH], F32, tag="rinvb")

        for jt in range(JT):
            kw = k_sb[:, :, jt * P:(jt + 1) * P]
            for half in range(2):
                ps = ps_big.tile([P, 1024], F32, tag="big",
                                 name=f"sc{jt}_{half}")
                for ii in range(2):
                    ib = 2 * half + ii
                    nc.tensor.matmul(
                        ps[:, ii * 512:(ii + 1) * 512], lhsT=kw,
                        rhs=q_sb[:, :, ib * 512:(ib + 1) * 512],
                        start=True, stop=True, perf_mode=DR)
                nc.scalar.activation(
                    out=eT[:, jt // 2, jt % 2, half * 1024:(half + 1) * 1024],
                    in_=ps, func=AF.Exp, scale=SCALE)

        # row sums -> DRAM-bounce broadcast -> fast reciprocal
        for ib in range(NB):
            i0 = ib * 512
            pss = ps_sum.tile([1, 512], F32, tag="sm", name=f"sm{ib}")
            for jtp in range(JP):
                nc.tensor.matmul(pss, lhsT=ones8,
                                 rhs=eT[:, jtp, :, i0:i0 + 512],
                                 start=(jtp == 0), stop=(jtp == JP - 1),
                                 perf_mode=DR)
            srow = pst.tile([1, 512], F32, tag="srow")
            nc.vector.tensor_copy(out=srow, in_=pss)
            nc.sync.dma_start(
                out=rinv_scr.ap().rearrange("(a b) -> a b", a=1)[:, i0:i0 + 512],
                in_=srow)
            rsc = rinv_scr.ap()[i0:i0 + 512]
            sb = pout.tile([P, 512], F32, tag="sb")
            nc.gpsimd.dma_start(
                out=sb,
                in_=bass.AP(tensor=rsc.tensor, offset=rsc.offset,
                            ap=[[0, P]] + [list(d) for d in rsc.ap]))
            nc.vector.reciprocal_approx_fast(out=rinvb[:, i0:i0 + 512], in_=sb)

        # PV: A_unnorm[c, i] += vT^T eT, vT slice stationary across all blocks
        for ct in range(CT):
            psas = [ps_big.tile([P, 1024], F32, tag="big", name=f"pv{ct}_{h}")
                    for h in range(2)]
            for jtp in range(JP):
                vw = vT_dr[:, :, jtp, ct * P:(ct + 1) * P]
                for half in range(2):
                    for ii in range(2):
                        ib = 2 * half + ii
                        nc.tensor.matmul(
                            psas[half][:, ii * 512:(ii + 1) * 512], lhsT=vw,
                            rhs=eT[:, jtp, :, ib * 512:(ib + 1) * 512],
                            start=(jtp == 0), stop=(jtp == JP - 1),
                            perf_mode=DR)
            for half in range(2):
                nc.vector.tensor_copy(
                    out=A_sb[ct][:, half * 1024:(half + 1) * 1024],
                    in_=psas[half])

        # ---- output projection + normalization + bias + residual ----
        for dt in range(CT):
            for icp in range(2):
                i0 = icp * 1024
                ps = ps_big.tile([P, 1024], F32, tag="big", name=f"pj{dt}_{icp}")
                for ii in range(2):
                    ic = 2 * icp + ii
                    for cc in range(CT):
                        nc.tensor.matmul(
                            ps[:, ii * 512:(ii + 1) * 512],
                            lhsT=w_sb[("wpT", cc)][:, dt * P:(dt + 1) * P],
                            rhs=A_sb[cc][:, ic * 512:(ic + 1) * 512],
                            start=(cc == 0), stop=(cc == CT - 1))
                ot = pout.tile([P, 1024], F32, tag="ot")
                nc.vector.tensor_mul(ot, ps, rinvb[:, i0:i0 + 1024])
                nc.vector.tensor_scalar(out=ot, in0=ot,
                                        scalar1=col_sb[("bpe", dt)],
                                        scalar2=None, op0=AluOpType.add)
                nc.vector.tensor_add(ot, ot, xh_sb[dt][:, i0:i0 + 1024])
                nc.sync.dma_start(
                    out=out.ap()[dt * P:(dt + 1) * P, i0:i0 + 1024],
                    in_=ot)


def _get_program():
    if "nc" not in _cache:
        _cache["nc"] = _build_program()
    return _cache["nc"]


def kernel(x, gn_scale, gn_bias, wq, bq, wk, bk, wv, bv, wproj, bproj):
    x = np.asarray(x, dtype=np.float32)
    b, c, hh, ww = x.shape
    assert (b, c, hh * ww) == (B, C, N)
    xf = np.ascontiguousarray(x.reshape(B, C, N))

    bf = ml_dtypes.bfloat16
    # Channel-pair interleave permutation for DoubleRow: even channels then odd.
    perm = np.concatenate([np.arange(0, C, 2), np.arange(1, C, 2)])
    wqT_s = np.ascontiguousarray(np.asarray(wq, np.float32).T[:, perm]).astype(bf)
    bq_s = np.ascontiguousarray(np.asarray(bq, np.float32)[perm])
    wkT = np.ascontiguousarray(np.asarray(wk, np.float32).T[:, perm]).astype(bf)
    bk_s = np.ascontiguousarray(np.asarray(bk, np.float32)[perm])
    wvT = np.ascontiguousarray(np.asarray(wv, np.float32).T[:, perm]).astype(bf)
    wpT = np.ascontiguousarray(np.asarray(wproj, np.float32).T[perm, :]).astype(bf)
    # softmax rows sum to 1 => v-bias contributes wproj@bv, constant per channel
    bpe = (np.asarray(bproj, np.float64)
           + np.asarray(wproj, np.float64) @ np.asarray(bv, np.float64)
           ).astype(np.float32)

    g8 = np.zeros((P, P // GSIZE), np.float32)
    gt01 = np.zeros((P // GSIZE, P), np.float32)
    for ch in range(P):
        g8[ch, ch // GSIZE] = 1.0 / GSIZE   # yields per-group means directly
        gt01[ch // GSIZE, ch] = 1.0

    common = dict(gnsc=np.asarray(gn_scale, np.float32),
                  gnbs=np.asarray(gn_bias, np.float32),
                  g8=g8, gt01=gt01,
                  wqT=wqT_s, bq=bq_s, wkT=wkT, bk=bk_s,
                  wvT=wvT, wpT=wpT, bpe=bpe)

    in_maps = []
    for core in range(NCORES):
        bi, half = core // 2, core % 2
        in_maps.append(dict(
            x_full=np.ascontiguousarray(xf[bi]),
            xh=np.ascontiguousarray(xf[bi][:, half * NH:(half + 1) * NH]),
            **common))

    nc = _get_program()
    trace = bool(os.environ.get("BASS_KERNEL_TRACE"))
    res = run_bass_kernel_spmd(nc, in_maps, core_ids=list(range(NCORES)),
                               trace=trace)
    _cache["last_results"] = res

    full = np.empty((B, C, N), np.float32)
    for core in range(NCORES):
        bi, half = core // 2, core % 2
        full[bi][:, half * NH:(half + 1) * NH] = res.results[core]["out"]
    return full.reshape(B, C, hh, ww)


# revision 15
# speedup vs baseline: 1.4470x; 1.0026x over previous
# Trainium2 Bass kernel for NonLocalBlock (GroupNorm + 1x1-conv self-attention + residual).
#
# Full input x: [4, 256, 64, 64] f32. Output: x + proj(attn(gn(x))), same shape.
#
# Sharding: 8 cores = 4 batches x 2 query-halves. Attention is independent per
# batch; within a batch, softmax rows (queries) split cleanly across 2 cores.
# Each core redundantly computes GroupNorm + K + V^T for its batch (cheap), and
# computes scores/softmax/PV/proj only for its 2048 queries. No collectives.
#
# Per-core program (c = 256 channels as 2 partition-tiles, n = 4096 keys):
#   - GroupNorm stats: bn_stats/bn_aggr per channel, group-combine and
#     broadcast-back via tiny PE matmuls with 0/1 group matrices.
#   - h = x*A + B (bf16), plus the query half from a separate input slice so
#     all access patterns stay static across the SPMD program.
#   - k, q, vT in fp8-e4m3 with the contraction dim stored channel-interleaved
#     ([128, 2, *]), so the attention matmuls run in DoubleRow perf mode
#     (2 fp8 MACs/cell/cycle, K=256 per instruction). The interleave is
#     produced for free: host permutes weight columns; PSUM->SBUF copies land
#     each output-channel half in its pair plane.
#   - scores transposed: sT[j,i] = k^T q; exp on ACT fused with the
#     PSUM->SBUF copy (1/sqrt(c) folded into the activation scale); eT[j,i]
#     is then directly the PV moving operand - no transposes anywhere.
#   - row sums of exp via ones-vector DR matmuls; softmax normalization is a
#     column scaling that commutes through PV and proj, applied in the output
#     stage (reciprocal_approx_fast on a broadcast of the sums).
#   - bv never applied on-chip: softmax rows sum to 1, so wproj@bv folds into
#     bproj on the host. out = x_half + rinv * (wproj @ A_unnorm) + bproj_eff.
#
# Stationary-operand reuse: each k/vT slice serves all 4 query blocks
# back-to-back, so LDWEIGHTS is paid once per 4 matmuls.

import os
import sys

for _p in ("/opt/trn_rl_repo", "/root/.axon_site/_ro/trn_rl_repo"):
    if os.path.isdir(_p) and _p not in sys.path:
        sys.path.insert(0, _p)

import numpy as np
import ml_dtypes

import concourse.bass as bass
import concourse.tile as tile
from concourse import bacc, mybir
from concourse.alu_op_type import AluOpType
from concourse.bass_utils import run_bass_kernel_spmd

F32 = mybir.dt.float32
BF16 = mybir.dt.bfloat16
F8 = mybir.dt.float8e4
AF = mybir.ActivationFunctionType
DR = mybir.MatmulPerfMode.DoubleRow

B = 4
C = 256
N = 4096           # 64*64 spatial positions
NH = N // 2        # queries per core
GROUPS = 32
GSIZE = C // GROUPS  # 8 channels per group
EPS = 1e-6
P = 128
CT = C // P        # 2 channel tiles
JT = N // P        # 32 key tiles (16 DoubleRow pairs)
JP = JT // 2
NB = NH // 512     # 4 query blocks of 512
NCORES = 8
SCALE = float(1.0 / np.sqrt(C))

_cache = {}


def _col(ap_1d, ct):
    # View a [256] DRAM tensor as [256, 1] and take channel-tile ct's [128, 1].
    return ap_1d.ap().rearrange("(a b) -> a b", b=1)[ct * P:(ct + 1) * P, :]


def _build_program():
    nc = bacc.Bacc("TRN2", target_bir_lowering=False, debug=False)

    x_full = nc.dram_tensor("x_full", [C, N], F32, kind="ExternalInput")
    xh = nc.dram_tensor("xh", [C, NH], F32, kind="ExternalInput")
    gnsc = nc.dram_tensor("gnsc", [C], F32, kind="ExternalInput")
    gnbs = nc.dram_tensor("gnbs", [C], F32, kind="ExternalInput")
    g8 = nc.dram_tensor("g8", [P, P // GSIZE], F32, kind="ExternalInput")
    gt01 = nc.dram_tensor("gt01", [P // GSIZE, P], F32, kind="ExternalInput")
    wqT = nc.dram_tensor("wqT", [C, C], BF16, kind="ExternalInput")
    bq = nc.dram_tensor("bq", [C], F32, kind="ExternalInput")
    wkT = nc.dram_tensor("wkT", [C, C], BF16, kind="ExternalInput")
    bk = nc.dram_tensor("bk", [C], F32, kind="ExternalInput")
    wvT = nc.dram_tensor("wvT", [C, C], BF16, kind="ExternalInput")
    wpT = nc.dram_tensor("wpT", [C, C], BF16, kind="ExternalInput")
    bpe = nc.dram_tensor("bpe", [C], F32, kind="ExternalInput")
    out = nc.dram_tensor("out", [C, NH], F32, kind="ExternalOutput")
    rinv_scr = nc.dram_tensor("rinv_scr", [NH], F32)

    with tile.TileContext(nc) as tc:
        _body(tc, x_full, xh, gnsc, gnbs, g8, gt01,
              wqT, bq, wkT, bk, wvT, wpT, bpe, out, rinv_scr)
    nc.compile()
    return nc


def _body(tc, x_full, xh, gnsc, gnbs, g8, gt01,
          wqT, bq, wkT, bk, wvT, wpT, bpe, out, rinv_scr):
    nc = tc.nc
    NG = P // GSIZE  # 16 groups per channel tile

    from contextlib import ExitStack
    with ExitStack() as ctx:
        consts = ctx.enter_context(tc.tile_pool(name="consts", bufs=1))
        px = ctx.enter_context(tc.tile_pool(name="px", bufs=1))
        ph = ctx.enter_context(tc.tile_pool(name="ph", bufs=1))
        pkv = ctx.enter_context(tc.tile_pool(name="pkv", bufs=1))
        pst = ctx.enter_context(tc.tile_pool(name="pst", bufs=4))
        pout = ctx.enter_context(tc.tile_pool(name="pout", bufs=3))
        # PSUM: two 2-bank score/misc slots + two 2-bank PV accumulators = 8
        ps_big = ctx.enter_context(tc.tile_pool(name="ps_big", bufs=2, space="PSUM"))
        ps_sum = ps_big

        # ---- x load first: one 1MB DMA per (ct, half), two queues ----
        x_sb = []
        for ct in range(CT):
            xt = px.tile([P, N], F32, tag=f"x{ct}", name=f"x{ct}")
            for c2 in range(2):
                [nc.sync, nc.scalar][(ct + c2) % 2].dma_start(
                    out=xt[:, c2 * 2048:(c2 + 1) * 2048],
                    in_=x_full.ap()[ct * P:(ct + 1) * P, c2 * 2048:(c2 + 1) * 2048])
            x_sb.append(xt)

        # ---- constants (gpsimd queue; keeps x queues clear) ----
        # DR weights need 16B-aligned pair-plane step; pad the ones vector
        ones8_t = consts.tile([P, 2, 16], F8, tag="ones")
        nc.vector.memset(ones8_t, 1.0)
        ones8 = ones8_t[:, :, 0:1]
        g8_sb = consts.tile([P, NG], F32, tag="g8")
        nc.gpsimd.dma_start(out=g8_sb, in_=g8.ap())
        gt01_sb = consts.tile([NG, P], F32, tag="gt01")
        nc.gpsimd.dma_start(out=gt01_sb, in_=gt01.ap())

        w_sb = {}
        for name, h in (("wqT", wqT), ("wkT", wkT), ("wvT", wvT), ("wpT", wpT)):
            for ec in range(CT):
                t = consts.tile([P, C], BF16, tag=f"{name}{ec}")
                nc.gpsimd.dma_start(out=t, in_=h.ap()[ec * P:(ec + 1) * P, :])
                w_sb[(name, ec)] = t

        col_sb = {}
        for name, h in (("gnsc", gnsc), ("gnbs", gnbs), ("bq", bq),
                        ("bk", bk), ("bpe", bpe)):
            for ct in range(CT):
                t = consts.tile([P, 1], F32, tag=f"{name}{ct}")
                nc.gpsimd.dma_start(out=t, in_=_col(h, ct))
                col_sb[(name, ct)] = t

        # ---- GroupNorm stats ----
        ab_cols = []
        for ct in range(CT):
            xt = x_sb[ct]
            stats = pst.tile([P, 8, nc.vector.BN_STATS_DIM], F32, tag="bnst")
            for s in range(8):
                nc.vector.bn_stats(out=stats[:, s, :], in_=xt[:, s * 512:(s + 1) * 512])
            mv = pst.tile([P, nc.vector.BN_AGGR_DIM], F32, tag="bnagg")
            nc.vector.bn_aggr(out=mv, in_=stats)

            # per-channel (mean, E[x^2]) -> per-group via G/8 matmul
            st2 = pst.tile([P, 2], F32, tag="st2")
            nc.vector.tensor_copy(out=st2[:, 0:1], in_=mv[:, 0:1])
            m2 = pst.tile([P, 1], F32, tag="m2")
            nc.vector.tensor_mul(m2, mv[:, 0:1], mv[:, 0:1])
            nc.vector.tensor_add(st2[:, 1:2], m2, mv[:, 1:2])

            gps = ps_big.tile([NG, 2], F32, tag="big")
            nc.tensor.matmul(gps, lhsT=g8_sb, rhs=st2, start=True, stop=True)
            gs = pst.tile([NG, 2], F32, tag="gs")
            nc.vector.tensor_copy(out=gs, in_=gps)

            # var_g = E[x^2]_g - mean_g^2 ; rstd = 1/sqrt(var+eps)
            vg = pst.tile([NG, 1], F32, tag="vg")
            nc.vector.tensor_mul(vg, gs[:, 0:1], gs[:, 0:1])
            nc.vector.tensor_tensor(out=vg, in0=gs[:, 1:2], in1=vg,
                                    op=AluOpType.subtract)
            eps_t = pst.tile([NG, 1], F32, tag="eps")
            nc.vector.memset(eps_t, EPS)
            std = pst.tile([NG, 1], F32, tag="std")
            nc.scalar.activation(out=std, in_=vg, func=AF.Sqrt, bias=eps_t, scale=1.0)
            rstd = pst.tile([NG, 1], F32, tag="rstd")
            nc.vector.reciprocal(out=rstd, in_=std)

            gs2 = pst.tile([NG, 2], F32, tag="gs2")
            nc.vector.tensor_copy(out=gs2[:, 0:1], in_=gs[:, 0:1])
            nc.vector.tensor_copy(out=gs2[:, 1:2], in_=rstd)

            # broadcast (mean_g, rstd_g) back to channels
            bps = ps_big.tile([P, 2], F32, tag="big")
            nc.tensor.matmul(bps, lhsT=gt01_sb, rhs=gs2, start=True, stop=True)
            mr = pst.tile([P, 2], F32, tag="mr")
            nc.vector.tensor_copy(out=mr, in_=bps)

            a_col = pst.tile([P, 1], F32, tag=f"acol{ct}")
            nc.vector.tensor_mul(a_col, mr[:, 1:2], col_sb[("gnsc", ct)])
            b_col = pst.tile([P, 1], F32, tag=f"bcol{ct}")
            nc.vector.tensor_mul(b_col, mr[:, 0:1], a_col)
            nc.vector.tensor_tensor(out=b_col, in0=col_sb[("gnbs", ct)],
                                    in1=b_col, op=AluOpType.subtract)
            ab_cols.append((a_col, b_col))

        # ---- h = x*A+B (chunked so k/vT matmuls start early); k, vT ----
        # k_sb/q_sb/vT_dr are fp8 with channels pair-interleaved for DoubleRow:
        # value (p, q, .) = channel 2p+q (host permuted the weight columns).
        h_sb = [ph.tile([P, N], BF16, tag=f"h{ct}", name=f"h{ct}") for ct in range(CT)]
        k_sb = pkv.tile([P, 2, N], F8, tag="k")
        vT_dr = pkv.tile([P, 2, JP, C], F8, tag="vT")
        for c4 in range(4):
            j0 = c4 * 1024
            for ct in range(CT):
                a_col, b_col = ab_cols[ct]
                nc.vector.tensor_scalar(
                    out=h_sb[ct][:, j0:j0 + 1024], in0=x_sb[ct][:, j0:j0 + 1024],
                    scalar1=a_col, scalar2=b_col,
                    op0=AluOpType.mult, op1=AluOpType.add)
            for dt in range(CT):
                ps = ps_big.tile([P, 1024], F32, tag=["big", "pva"][(c4 + dt) % 2], name=f"k{c4}_{dt}")
                for jj in range(2):
                    jc = 2 * c4 + jj
                    for ec in range(CT):
                        nc.tensor.matmul(
                            ps[:, jj * 512:(jj + 1) * 512],
                            lhsT=w_sb[("wkT", ec)][:, dt * P:(dt + 1) * P],
                            rhs=h_sb[ec][:, jc * 512:(jc + 1) * 512],
                            start=(ec == 0), stop=(ec == CT - 1))
                nc.scalar.activation(
                    out=k_sb[:, dt, j0:j0 + 1024], in_=ps,
                    func=AF.Identity, bias=col_sb[("bk", dt)], scale=1.0)
            for jt in range(8 * c4, 8 * c4 + 8):
                ps = ps_big.tile([P, 1024], F32, tag=["big", "pva"][jt % 2], name=f"v{jt}")
                for ec in range(CT):
                    nc.tensor.matmul(
                        ps[:, 0:C], lhsT=h_sb[ec][:, jt * P:(jt + 1) * P],
                        rhs=w_sb[("wvT", ec)],
                        start=(ec == 0), stop=(ec == CT - 1))
                nc.scalar.activation(out=vT_dr[:, jt % 2, jt // 2, :],
                                     in_=ps[:, 0:C], func=AF.Copy)

        # ---- query-half h, q ----
        xh_sb, hh_sb = [], []
        for ct in range(CT):
            xht = px.tile([P, NH], F32, tag=f"x{ct}", name=f"xh{ct}")
            [nc.sync, nc.scalar][ct].dma_start(
                out=xht, in_=xh.ap()[ct * P:(ct + 1) * P, :])
            xh_sb.append(xht)
            a_col, b_col = ab_cols[ct]
            hht = ph.tile([P, NH], BF16, tag=f"hh{ct}", name=f"hh{ct}")
            nc.vector.tensor_scalar(out=hht, in0=xht, scalar1=a_col, scalar2=b_col,
                                    op0=AluOpType.mult, op1=AluOpType.add)
            hh_sb.append(hht)

        q_sb = pkv.tile([P, 2, NH], F8, tag="q")
        for dt in range(CT):
            for icp in range(2):
                ps = ps_big.tile([P, 1024], F32, tag=["big", "pva"][(dt + icp) % 2], name=f"q{dt}_{icp}")
                for ii in range(2):
                    ic = 2 * icp + ii
                    for ec in range(CT):
                        nc.tensor.matmul(
                            ps[:, ii * 512:(ii + 1) * 512],
                            lhsT=w_sb[("wqT", ec)][:, dt * P:(dt + 1) * P],
                            rhs=hh_sb[ec][:, ic * 512:(ic + 1) * 512],
                            start=(ec == 0), stop=(ec == CT - 1))
                nc.scalar.activation(
                    out=q_sb[:, dt, icp * 1024:(icp + 1) * 1024], in_=ps,
                    func=AF.Identity, bias=col_sb[("bq", dt)], scale=1.0)

        # ---- attention: all 2048 queries in one pass over the 32 key tiles.
        # eT[p, jtp, q, i] = exp(s[j=(2*jtp+q)*128+p, i] / 16)  (fp8) ----
        eT = pkv.tile([P, JP, 2, NH], F8, tag="eT")
        A_sb = [pkv.tile([P, NH], BF16, tag=f"A{ct}", name=f"A{ct}")
                for ct in range(CT)]
        rinvb = pkv.tile([P, NH], F32, tag="rinvb")

        pv0 = [ps_big.tile([P, 1024], F32, tag="pva", name=f"pv0_{h}")
               for h in range(2)]
        for jt in range(JT):
            kw = k_sb[:, :, jt * P:(jt + 1) * P]
            for half in range(2):
                ps = ps_big.tile([P, 1024], F32, tag="big",
                                 name=f"sc{jt}_{half}")
                for ii in range(2):
                    ib = 2 * half + ii
                    nc.tensor.matmul(
                        ps[:, ii * 512:(ii + 1) * 512], lhsT=kw,
                        rhs=q_sb[:, :, ib * 512:(ib + 1) * 512],
                        start=True, stop=True, perf_mode=DR)
                nc.scalar.activation(
                    out=eT[:, jt // 2, jt % 2, half * 1024:(half + 1) * 1024],
                    in_=ps, func=AF.Exp, scale=SCALE)
            if jt % 2 == 1:
                # PV for channel-tile 0 rides along as soon as a jt-pair of
                # exp output lands; keeps PE fed while ACT works on exp
                jtp = jt // 2
                vw = vT_dr[:, :, jtp, 0:P]
                for half in range(2):
                    for ii in range(2):
                        ib = 2 * half + ii
                        nc.tensor.matmul(
                            pv0[half][:, ii * 512:(ii + 1) * 512], lhsT=vw,
                            rhs=eT[:, jtp, :, ib * 512:(ib + 1) * 512],
                            start=(jtp == 0), stop=(jtp == JP - 1),
                            perf_mode=DR)
        for half in range(2):
            nc.scalar.activation(
                out=A_sb[0][:, half * 1024:(half + 1) * 1024],
                in_=pv0[half], func=AF.Copy)

        # row sums -> DRAM-bounce broadcast -> fast reciprocal
        for ib in range(NB):
            i0 = ib * 512
            pss = ps_sum.tile([1, 512], F32, tag="big", name=f"sm{ib}")
            for jtp in range(JP):
                nc.tensor.matmul(pss, lhsT=ones8,
                                 rhs=eT[:, jtp, :, i0:i0 + 512],
                                 start=(jtp == 0), stop=(jtp == JP - 1),
                                 perf_mode=DR)
            srow = pst.tile([1, 512], F32, tag="srow")
            nc.vector.tensor_copy(out=srow, in_=pss)
            nc.sync.dma_start(
                out=rinv_scr.ap().rearrange("(a b) -> a b", a=1)[:, i0:i0 + 512],
                in_=srow)
            rsc = rinv_scr.ap()[i0:i0 + 512]
            sb = pout.tile([P, 512], F32, tag="sb")
            nc.gpsimd.dma_start(
                out=sb,
                in_=bass.AP(tensor=rsc.tensor, offset=rsc.offset,
                            ap=[[0, P]] + [list(d) for d in rsc.ap]))
            nc.vector.reciprocal_approx_fast(out=rinvb[:, i0:i0 + 512], in_=sb)

        # PV for channel-tile 1 (ct 0 was interleaved with scores above)
        psas = [ps_big.tile([P, 1024], F32, tag="pva", name=f"pv1_{h}")
                for h in range(2)]
        for jtp in range(JP):
            vw = vT_dr[:, :, jtp, P:2 * P]
            for half in range(2):
                for ii in range(2):
                    ib = 2 * half + ii
                    nc.tensor.matmul(
                        psas[half][:, ii * 512:(ii + 1) * 512], lhsT=vw,
                        rhs=eT[:, jtp, :, ib * 512:(ib + 1) * 512],
                        start=(jtp == 0), stop=(jtp == JP - 1),
                        perf_mode=DR)
        for half in range(2):
            nc.scalar.activation(
                out=A_sb[1][:, half * 1024:(half + 1) * 1024],
                in_=psas[half], func=AF.Copy)

        # ---- output projection + normalization + bias + residual ----
        for dt in range(CT):
            for icp in range(2):
                i0 = icp * 1024
                ps = ps_big.tile([P, 1024], F32, tag=["big", "pva"][(dt + icp) % 2], name=f"pj{dt}_{icp}")
                for ii in range(2):
                    ic = 2 * icp + ii
                    for cc in range(CT):
                        nc.tensor.matmul(
                            ps[:, ii * 512:(ii + 1) * 512],
                            lhsT=w_sb[("wpT", cc)][:, dt * P:(dt + 1) * P],
                            rhs=A_sb[cc][:, ic * 512:(ic + 1) * 512],
                            start=(cc == 0), stop=(cc == CT - 1))
                ot = pout.tile([P, 1024], F32, tag="ot")
                nc.vector.tensor_mul(ot, ps, rinvb[:, i0:i0 + 1024])
                nc.vector.tensor_scalar(out=ot, in0=ot,
                                        scalar1=col_sb[("bpe", dt)],
                                        scalar2=None, op0=AluOpType.add)
                nc.vector.tensor_add(ot, ot, xh_sb[dt][:, i0:i0 + 1024])
                nc.sync.dma_start(
                    out=out.ap()[dt * P:(dt + 1) * P, i0:i0 + 1024],
                    in_=ot)


def _get_program():
    if "nc" not in _cache:
        _cache["nc"] = _build_program()
    return _cache["nc"]


def kernel(x, gn_scale, gn_bias, wq, bq, wk, bk, wv, bv, wproj, bproj):
    x = np.asarray(x, dtype=np.float32)
    b, c, hh, ww = x.shape
    assert (b, c, hh * ww) == (B, C, N)
    xf = np.ascontiguousarray(x.reshape(B, C, N))

    bf = ml_dtypes.bfloat16
    # Channel-pair interleave permutation for DoubleRow: even channels then odd.
    perm = np.concatenate([np.arange(0, C, 2), np.arange(1, C, 2)])
    wqT_s = np.ascontiguousarray(np.asarray(wq, np.float32).T[:, perm]).astype(bf)
    bq_s = np.ascontiguousarray(np.asarray(bq, np.float32)[perm])
    wkT = np.ascontiguousarray(np.asarray(wk, np.float32).T[:, perm]).astype(bf)
    bk_s = np.ascontiguousarray(np.asarray(bk, np.float32)[perm])
    wvT = np.ascontiguousarray(np.asarray(wv, np.float32).T[:, perm]).astype(bf)
    wpT = np.ascontiguousarray(np.asarray(wproj, np.float32).T[perm, :]).astype(bf)
    # softmax rows sum to 1 => v-bias contributes wproj@bv, constant per channel
    bpe = (np.asarray(bproj, np.float64)
           + np.asarray(wproj, np.float64) @ np.asarray(bv, np.float64)
           ).astype(np.float32)

    g8 = np.zeros((P, P // GSIZE), np.float32)
    gt01 = np.zeros((P // GSIZE, P), np.float32)
    for ch in range(P):
        g8[ch, ch // GSIZE] = 1.0 / GSIZE   # yields per-group means directly
        gt01[ch // GSIZE, ch] = 1.0

    common = dict(gnsc=np.asarray(gn_scale, np.float32),
                  gnbs=np.asarray(gn_bias, np.float32),
                  g8=g8, gt01=gt01,
                  wqT=wqT_s, bq=bq_s, wkT=wkT, bk=bk_s,
                  wvT=wvT, wpT=wpT, bpe=bpe)

    in_maps = []
    for core in range(NCORES):
        bi, half = core // 2, core % 2
        in_maps.append(dict(
            x_full=np.ascontiguousarray(xf[bi]),
            xh=np.ascontiguousarray(xf[bi][:, half * NH:(half + 1) * NH]),
            **common))

    nc = _get_program()
    trace = bool(os.environ.get("BASS_KERNEL_TRACE"))
    res = run_bass_kernel_spmd(nc, in_maps, core_ids=list(range(NCORES)),
                               trace=trace)
    _cache["last_results"] = res

    full = np.empty((B, C, N), np.float32)
    for core in range(NCORES):
        bi, half = core // 2, core % 2
        full[bi][:, half * NH:(half + 1) * NH] = res.results[core]["out"]
    return full.reshape(B, C, hh, ww)
